# revision 13
# baseline (speedup 1.0000x reference)
"""Trainium2 Bass kernel for CrossAttentionFusion.

Reference computation (per batch b):
    Q = q_w @ f1 + q_b          (O, N)   f1 = features1[b] as (C, N)
    K = k_w @ f2 + k_b          (O, N)
    V = v_w @ f2 + v_b          -> used as (N, O)
    A = softmax(Q^T K / sqrt(O))  over keys          (N, N)
    att = A @ V                  (N, O)
    Z = o_w @ att^T + o_b        (O, N)
    out = GroupNorm(8 groups over O, spatial N)(Z) * gn_w + gn_b

Sharding: pure data-parallel, batch b -> NeuronCore b (B=8, 8 cores).

Layout trick: scores are computed transposed (S' = K^T Q in [nk, nq] tiles)
so the exp output P' feeds the A@V matmul directly (lhsT = V tile, rhs = P')
with zero on-chip transposes of the big attention matrix. Softmax needs no
max-subtraction: scores ~ N(0,1), exp stays well inside fp32 range.
Denominators (cross-partition sums of P') come from a pairwise bf16 DVE tree
plus one ones-vector matmul; 1/denom is computed with the fast approximate
reciprocal and broadcast across partitions by a stride-0 DMA.
QK^T / projections run in float32r (fp32 storage, 1 cycle/row at free>=256);
P' and V are bf16 (DVE 4x mode, PE fast-weight-load). Weight matrices are
transposed host-side, so no on-chip weight transposes are needed.
"""

import numpy as np

B = 8
C = 256
O = 256
N = 2304
NKT = 18  # key tiles of 128
BIG = [(0, 1024), (1024, 1024), (2048, 256)]  # query chunks
GROUPS = 8
EPS = 1e-5
SCALE = float(O) ** -0.5

_BUILD_CACHE = {}


def _subs(jw):
    return [(s, min(512, jw - s)) for s in range(0, jw, 512)]


def _build_nc():
    import concourse.mybir as mybir
    import concourse.tile as tile
    from concourse import bacc
    from concourse.bass import ts

    F32 = mybir.dt.float32
    F32R = mybir.dt.float32r
    BF16 = mybir.dt.bfloat16
    AF = mybir.ActivationFunctionType
    ALU = mybir.AluOpType
    AX = mybir.AxisListType

    nc = bacc.Bacc("TRN2", target_bir_lowering=False)

    f1_d = nc.dram_tensor("features1", [C, N], F32R, kind="ExternalInput")
    f2_d = nc.dram_tensor("features2", [C, N], F32R, kind="ExternalInput")
    # host-pre-transposed weights
    qwT_d = nc.dram_tensor("qwT", [C, O], F32R, kind="ExternalInput")
    kwT_d = nc.dram_tensor("kwT", [C, O], F32R, kind="ExternalInput")
    vwT_d = nc.dram_tensor("vwT", [C, O], F32R, kind="ExternalInput")
    owT_d = nc.dram_tensor("owT", [O, O], F32R, kind="ExternalInput")
    qb_d = nc.dram_tensor("q_b", [O], F32, kind="ExternalInput")
    kb_d = nc.dram_tensor("k_b", [O], F32, kind="ExternalInput")
    vbb_d = nc.dram_tensor("vb_bcast", [128, O], F32, kind="ExternalInput")
    ob_d = nc.dram_tensor("o_b", [O], F32, kind="ExternalInput")
    gnw_d = nc.dram_tensor("gn_w", [O], F32, kind="ExternalInput")
    gnb_d = nc.dram_tensor("gn_b", [O], F32, kind="ExternalInput")
    gsel_d = nc.dram_tensor("gsel", [128, 2 * GROUPS], F32, kind="ExternalInput")
    gselT_d = nc.dram_tensor("gselT", [GROUPS, 2 * 128], F32, kind="ExternalInput")
    onesb_d = nc.dram_tensor("ones_bf", [128, 1], BF16, kind="ExternalInput")
    onesr_d = nc.dram_tensor("ones_row_r", [1, 128], F32R, kind="ExternalInput")
    out_d = nc.dram_tensor("out", [O, N], F32, kind="ExternalOutput")

    with tile.TileContext(nc) as tc:
        with (
            tc.tile_pool(name="consts", bufs=1) as consts,
            tc.tile_pool(name="weights", bufs=1) as wpool,
            tc.tile_pool(name="acts", bufs=1) as apool,
        ):
            # ---- feature loads first: they gate the first matmuls.
            # Strip-split so all 16 DMA queues pull in parallel (one queue
            # moves only ~25 GB/s of 2KB packets). f2 first: K gates QK^T.
            f1sb = [apool.tile([128, N], F32R, name=f"f1sb{t}") for t in range(2)]
            f2sb = [apool.tile([128, N], F32R, name=f"f2sb{t}") for t in range(2)]
            fstrips = [
                (p0, c0, min(512, N - c0))
                for c0 in range(0, N, 512)
                for p0 in range(0, 128, 32)
            ]
            for sb, dr in ((f2sb, f2_d), (f1sb, f1_d)):
                for t in range(2):
                    for p0, c0, cw in fstrips:
                        nc.sync.dma_start(
                            out=sb[t][p0 : p0 + 32, c0 : c0 + cw],
                            in_=dr[128 * t + p0 : 128 * t + p0 + 32, c0 : c0 + cw],
                        )

            # ---- constants ----
            zero_col = consts.tile([128, 1], F32, name="zero_col")
            nc.vector.memset(zero_col, 0.0)
            ones_bf = consts.tile([128, 1], BF16, name="ones_bf")
            nc.sync.dma_start(out=ones_bf, in_=onesb_d[:, :])
            ones_row = consts.tile([1, 128], F32R, name="ones_row")
            nc.sync.dma_start(out=ones_row, in_=onesr_d[:, :])
            vb_bc = consts.tile([128, O], F32, name="vb_bc")
            nc.sync.dma_start(out=vb_bc, in_=vbb_d[:, :])
            gsel = consts.tile([128, 2 * GROUPS], F32, name="gsel")
            nc.sync.dma_start(out=gsel, in_=gsel_d[:, :])
            gselT = consts.tile([GROUPS, 2 * 128], F32, name="gselT")
            nc.sync.dma_start(out=gselT, in_=gselT_d[:, :])

            qb_c, kb_c, ob_c, gnw_c, gnb_c = [], [], [], [], []
            for t in range(2):
                for lst, src, nm in (
                    (qb_c, qb_d, "qb"),
                    (kb_c, kb_d, "kb"),
                    (ob_c, ob_d, "ob"),
                    (gnw_c, gnw_d, "gnw"),
                    (gnb_c, gnb_d, "gnb"),
                ):
                    col = consts.tile([128, 1], F32, name=f"{nm}{t}")
                    nc.sync.dma_start(out=col, in_=src[ts(t, 128)].unsqueeze(1))
                    lst.append(col)

            # ---- persistent weights / activations ----
            qwT = [wpool.tile([128, O], F32R, name=f"qwT{t}") for t in range(2)]
            kwT = [wpool.tile([128, O], F32R, name=f"kwT{t}") for t in range(2)]
            vwT = [wpool.tile([128, O], F32R, name=f"vwT{t}") for t in range(2)]
            owT = [wpool.tile([128, O], F32R, name=f"owT{t}") for t in range(2)]
            for wt, wd in ((kwT, kwT_d), (vwT, vwT_d), (qwT, qwT_d), (owT, owT_d)):
                for t in range(2):
                    for p0 in range(0, 128, 64):
                        nc.sync.dma_start(
                            out=wt[t][p0 : p0 + 64, :],
                            in_=wd[128 * t + p0 : 128 * t + p0 + 64, :],
                        )
            Q = [apool.tile([128, N], F32R, name=f"Q{t}") for t in range(2)]
            K = [apool.tile([128, N], F32R, name=f"K{t}") for t in range(2)]
            V = [apool.tile([128, O], BF16, name=f"V{k}") for k in range(NKT)]
            Z = [apool.tile([128, N], F32, name=f"Z{t}") for t in range(2)]
            # running GroupNorm stats per p-tile: col0 = sum, col1 = sumsq
            st_sums = [apool.tile([128, 2], F32, name=f"st{t}") for t in range(2)]
            for t in range(2):
                nc.vector.memset(st_sums[t], 0.0)

            # ---- phase 1: Q/K/V projections ----
            with (
                tc.tile_pool(name="fch", bufs=3) as fpool,
                tc.tile_pool(name="pps", bufs=4, space="PSUM") as pps,
            ):
                pchunks = ((0, 512), (512, 512), (1024, 512), (1536, 512), (2048, 256))
                for j0, jw in pchunks:
                    fa = fpool.tile([128, jw], F32R, tag="fa", name="fa")
                    fb = fpool.tile([128, jw], F32R, tag="fb", name="fb")
                    nc.sync.dma_start(out=fa, in_=f1_d[0:128, j0 : j0 + jw])
                    nc.sync.dma_start(out=fb, in_=f1_d[128:256, j0 : j0 + jw])
                    for t in range(2):
                        qp = pps.tile([128, jw], F32, tag="pp", name="qp")
                        nc.tensor.matmul(
                            qp, qwT[0][:, ts(t, 128)], fa, start=True, stop=False
                        )
                        nc.tensor.matmul(
                            qp, qwT[1][:, ts(t, 128)], fb, start=False, stop=True
                        )
                        nc.vector.tensor_scalar_add(
                            Q[t][:, j0 : j0 + jw], qp, qb_c[t]
                        )
                for j0, jw in pchunks:
                    fa = fpool.tile([128, jw], F32R, tag="fa", name="fa")
                    fb = fpool.tile([128, jw], F32R, tag="fb", name="fb")
                    nc.sync.dma_start(out=fa, in_=f2_d[0:128, j0 : j0 + jw])
                    nc.sync.dma_start(out=fb, in_=f2_d[128:256, j0 : j0 + jw])
                    for t in range(2):
                        kp = pps.tile([128, jw], F32, tag="pp", name="kp")
                        nc.tensor.matmul(
                            kp, kwT[0][:, ts(t, 128)], fa, start=True, stop=False
                        )
                        nc.tensor.matmul(
                            kp, kwT[1][:, ts(t, 128)], fb, start=False, stop=True
                        )
                        nc.vector.tensor_scalar_add(
                            K[t][:, j0 : j0 + jw], kp, kb_c[t]
                        )
                    for s in range(jw // 128):
                        nk = j0 // 128 + s
                        vp = pps.tile([128, O], F32, tag="pp", name="vp")
                        nc.tensor.matmul(
                            vp, fa[:, ts(s, 128)], vwT[0], start=True, stop=False
                        )
                        nc.tensor.matmul(
                            vp, fb[:, ts(s, 128)], vwT[1], start=False, stop=True
                        )
                        nc.vector.tensor_add(V[nk], vp, vb_bc)

            # ---- phase 2: attention + output projection, per query chunk ----
            with (
                tc.tile_pool(name="ppool", bufs=1) as ppool,
                tc.tile_pool(name="tpool", bufs=1) as tpool,
                tc.tile_pool(name="sbm", bufs=2) as sbm,
                tc.tile_pool(name="sps", bufs=2, space="PSUM") as sps,
                tc.tile_pool(name="ops", bufs=2, space="PSUM") as ops,
                tc.tile_pool(name="zps", bufs=1, space="PSUM") as zps,
                tc.tile_pool(name="dps", bufs=1, space="PSUM") as dps,
            ):
                for j0, jw in BIG:
                    subs = _subs(jw)
                    # scores (transposed) + exp, batched over the whole chunk
                    P = []
                    for nk in range(NKT):
                        sp = sps.tile([128, 1024], F32, tag="sp", name="sp")
                        for s0, sw in subs:
                            nc.tensor.matmul(
                                sp[:, s0 : s0 + sw],
                                K[0][:, ts(nk, 128)],
                                Q[0][:, j0 + s0 : j0 + s0 + sw],
                                start=True,
                                stop=False,
                            )
                            nc.tensor.matmul(
                                sp[:, s0 : s0 + sw],
                                K[1][:, ts(nk, 128)],
                                Q[1][:, j0 + s0 : j0 + s0 + sw],
                                start=False,
                                stop=True,
                            )
                        pt = ppool.tile([128, 1024], BF16, tag=f"p{nk}", name=f"pt{nk}")
                        nc.scalar.activation(
                            pt[:, :jw], sp[:, :jw], AF.Exp, bias=zero_col, scale=SCALE
                        )
                        P.append(pt)
                    # denominator: pairwise bf16 tree over the 18 P' tiles
                    tr = [
                        tpool.tile([128, 1024], BF16, tag=f"tr{i}", name=f"tr{i}")
                        for i in range(9)
                    ]
                    for i in range(9):
                        nc.vector.tensor_add(
                            tr[i][:, :jw], P[2 * i][:, :jw], P[2 * i + 1][:, :jw]
                        )
                    for i in range(4):
                        nc.vector.tensor_add(
                            tr[2 * i][:, :jw], tr[2 * i][:, :jw], tr[2 * i + 1][:, :jw]
                        )
                    nc.vector.tensor_add(tr[0][:, :jw], tr[0][:, :jw], tr[2][:, :jw])
                    nc.vector.tensor_add(tr[4][:, :jw], tr[4][:, :jw], tr[6][:, :jw])
                    nc.vector.tensor_add(tr[0][:, :jw], tr[0][:, :jw], tr[4][:, :jw])
                    nc.vector.tensor_add(tr[0][:, :jw], tr[0][:, :jw], tr[8][:, :jw])
                    for s0, sw in subs:
                        ssl = slice(s0, s0 + sw)
                        dn = dps.tile([1, 512], F32, tag="d", name="dn")
                        nc.tensor.matmul(
                            dn[:, :sw], ones_bf, tr[0][:, ssl], start=True, stop=True
                        )
                        dnr = sbm.tile([1, 512], F32R, tag="dnr", name="dnr")
                        nc.scalar.activation(dnr[:, :sw], dn[:, :sw], AF.Copy)
                        bc = dps.tile([128, 512], F32, tag="d", name="bc")
                        nc.tensor.matmul(
                            bc[:, :sw], ones_row, dnr[:, :sw], start=True, stop=True
                        )
                        bcs = sbm.tile([128, 512], F32, tag="bcs", name="bcs")
                        nc.vector.reciprocal_approx_fast(bcs[:, :sw], bc[:, :sw])
                        # att^T sub-chunk = (V^T P') * (1/denom) : [O, sw]
                        ATs = []
                        for o in range(2):
                            op = ops.tile([128, 512], F32, tag="op", name="op")
                            for nk in range(NKT):
                                nc.tensor.matmul(
                                    op[:, :sw],
                                    V[nk][:, ts(o, 128)],
                                    P[nk][:, ssl],
                                    start=(nk == 0),
                                    stop=(nk == NKT - 1),
                                )
                            at = sbm.tile([128, 512], F32R, tag=f"at{o}", name=f"at{o}")
                            nc.vector.tensor_mul(at[:, :sw], op[:, :sw], bcs[:, :sw])
                            ATs.append(at)
                        # output projection sub-chunk: Z[p, sw]
                        zsl = slice(j0 + s0, j0 + s0 + sw)
                        for p in range(2):
                            zp = zps.tile([128, 512], F32, tag="zp", name="zp")
                            nc.tensor.matmul(
                                zp[:, :sw],
                                owT[0][:, ts(p, 128)],
                                ATs[0][:, :sw],
                                start=True,
                                stop=False,
                            )
                            nc.tensor.matmul(
                                zp[:, :sw],
                                owT[1][:, ts(p, 128)],
                                ATs[1][:, :sw],
                                start=False,
                                stop=True,
                            )
                            # evacuate + bias; accum_out gives GN row-sums free
                            part = sbm.tile(
                                [128, 2], F32, tag=f"part{p}", name=f"part{p}"
                            )
                            nc.vector.tensor_scalar(
                                Z[p][:, zsl],
                                zp[:, :sw],
                                ob_c[p],
                                0.0,
                                op0=ALU.add,
                                op1=ALU.add,
                                accum_out=part[:, 0:1],
                            )
                            sqs = sbm.tile([128, 512], F32, tag="sqs", name="sqs")
                            nc.scalar.activation(
                                sqs[:, :sw],
                                Z[p][:, zsl],
                                AF.Square,
                                bias=zero_col,
                                accum_out=part[:, 1:2],
                            )
                            nc.vector.tensor_add(st_sums[p], st_sums[p], part)

            # ---- phase 3: GroupNorm finalization ----
            with (
                tc.tile_pool(name="gns", bufs=2) as gns,
                tc.tile_pool(name="gout", bufs=2) as gout,
                tc.tile_pool(name="gps", bufs=2, space="PSUM") as gps,
            ):
                gst = gps.tile([GROUPS, 2], F32, tag="gst", name="gst")
                nc.tensor.matmul(
                    gst, gsel[:, 0:GROUPS], st_sums[0], start=True, stop=False
                )
                nc.tensor.matmul(
                    gst,
                    gsel[:, GROUPS : 2 * GROUPS],
                    st_sums[1],
                    start=False,
                    stop=True,
                )
                # per-group mean / rstd on 8 partitions
                mv = gns.tile([GROUPS, 8], F32, tag="mv", name="mv")
                inv_cnt = 1.0 / (32.0 * N)
                nc.vector.tensor_scalar_mul(mv[:, 0:1], gst[:, 0:1], inv_cnt)  # mean
                nc.vector.tensor_scalar_mul(mv[:, 1:2], gst[:, 1:2], inv_cnt)  # E[x^2]
                nc.vector.tensor_mul(mv[:, 2:3], mv[:, 0:1], mv[:, 0:1])  # mean^2
                nc.vector.tensor_sub(mv[:, 3:4], mv[:, 1:2], mv[:, 2:3])  # var
                nc.vector.tensor_scalar_add(mv[:, 3:4], mv[:, 3:4], EPS)  # var+eps
                nc.scalar.activation(
                    mv[:, 4:5], mv[:, 3:4], AF.Sqrt, bias=zero_col[0:GROUPS, :]
                )
                nc.vector.reciprocal(mv[:, 5:6], mv[:, 4:5])  # y0 ~ rsqrt
                # one Newton step: y1 = y0 * (1.5 - 0.5*(var+eps)*y0^2)
                nc.vector.tensor_mul(mv[:, 6:7], mv[:, 5:6], mv[:, 5:6])  # y0^2
                nc.vector.tensor_mul(mv[:, 6:7], mv[:, 6:7], mv[:, 3:4])  # v*y0^2
                nc.vector.tensor_scalar(
                    mv[:, 6:7], mv[:, 6:7], -0.5, 1.5, op0=ALU.mult, op1=ALU.add
                )
                nc.vector.tensor_mul(mv[:, 7:8], mv[:, 5:6], mv[:, 6:7])  # rstd
                gm2 = gns.tile([GROUPS, 2], F32, tag="gm2", name="gm2")
                nc.vector.tensor_copy(gm2[:, 0:1], mv[:, 0:1])
                nc.vector.tensor_copy(gm2[:, 1:2], mv[:, 7:8])
                for p in range(2):
                    pst = gps.tile([128, 2], F32, tag="pst", name="pst")
                    nc.tensor.matmul(
                        pst, gselT[:, ts(p, 128)], gm2, start=True, stop=True
                    )
                    a_col = gns.tile([128, 1], F32, tag="a_col", name="a_col")
                    nc.vector.tensor_mul(a_col, pst[:, 1:2], gnw_c[p])
                    t_col = gns.tile([128, 1], F32, tag="t_col", name="t_col")
                    nc.vector.tensor_mul(t_col, pst[:, 0:1], a_col)
                    b_col = gns.tile([128, 1], F32, tag="b_col", name="b_col")
                    nc.vector.tensor_sub(b_col, gnb_c[p], t_col)
                    # scale+shift in column halves so output DMA overlaps compute
                    for h0 in (0, 1152):
                        outp = gout.tile([128, 1152], F32, tag="outp", name="outp")
                        nc.vector.tensor_scalar(
                            outp,
                            Z[p][:, h0 : h0 + 1152],
                            a_col,
                            b_col,
                            op0=ALU.mult,
                            op1=ALU.add,
                        )
                        nc.sync.dma_start(
                            out=out_d[ts(p, 128), h0 : h0 + 1152], in_=outp
                        )

    nc.finalize()
    return nc


def _get_nc():
    if "nc" not in _BUILD_CACHE:
        _BUILD_CACHE["nc"] = _build_nc()
    return _BUILD_CACHE["nc"]


def _make_in_maps(inputs):
    import ml_dtypes

    f1 = np.ascontiguousarray(
        np.asarray(inputs["features1"], dtype=np.float32).reshape(B, C, N)
    )
    f2 = np.ascontiguousarray(
        np.asarray(inputs["features2"], dtype=np.float32).reshape(B, C, N)
    )

    def g(k):
        return np.asarray(inputs[k], dtype=np.float32)

    gsel = np.zeros((128, 2 * GROUPS), np.float32)
    gselT = np.zeros((GROUPS, 2 * 128), np.float32)
    for t in range(2):
        for gl in range(4):
            grp = 4 * t + gl
            gsel[gl * 32 : (gl + 1) * 32, GROUPS * t + grp] = 1.0
            gselT[grp, 128 * t + gl * 32 : 128 * t + (gl + 1) * 32] = 1.0
    shared = {
        "qwT": np.ascontiguousarray(g("q_w").T),
        "kwT": np.ascontiguousarray(g("k_w").T),
        "vwT": np.ascontiguousarray(g("v_w").T),
        "owT": np.ascontiguousarray(g("o_w").T),
        "q_b": g("q_b"),
        "k_b": g("k_b"),
        "vb_bcast": np.ascontiguousarray(np.tile(g("v_b")[None, :], (128, 1))),
        "o_b": g("o_b"),
        "gn_w": g("gn_w"),
        "gn_b": g("gn_b"),
        "gsel": gsel,
        "gselT": gselT,
        "ones_bf": np.ones((128, 1), ml_dtypes.bfloat16),
        "ones_row_r": np.ones((1, 128), np.float32),
    }
    return [{"features1": f1[i], "features2": f2[i], **shared} for i in range(B)]


def run(inputs, trace=False):
    from concourse.bass_utils import run_bass_kernel_spmd

    nc = _get_nc()
    in_maps = _make_in_maps(inputs)
    res = run_bass_kernel_spmd(nc, in_maps, core_ids=list(range(B)), trace=trace)
    out = np.stack([np.asarray(res.results[i]["out"]) for i in range(B)])
    return out.reshape(B, O, 48, 48).astype(np.float32), res


def kernel(**inputs):
    out, _ = run(inputs, trace=False)
    return out


# revision 15
# speedup vs baseline: 1.2766x; 1.2766x over previous
"""Trainium2 Bass kernel for CrossAttentionFusion.

Reference computation (per batch b):
    Q = q_w @ f1 + q_b          (O, N)   f1 = features1[b] as (C, N)
    K = k_w @ f2 + k_b          (O, N)
    V = v_w @ f2 + v_b          -> used as (N, O)
    A = softmax(Q^T K / sqrt(O))  over keys          (N, N)
    att = A @ V                  (N, O)
    Z = o_w @ att^T + o_b        (O, N)
    out = GroupNorm(8 groups over O, spatial N)(Z) * gn_w + gn_b

Sharding: pure data-parallel, batch b -> NeuronCore b (B=8, 8 cores).

Layout trick: scores are computed transposed (S' = K^T Q in [nk, nq] tiles)
so the exp output P' feeds the A@V matmul directly (lhsT = V tile, rhs = P')
with zero on-chip transposes of the big attention matrix. Softmax needs no
max-subtraction: scores ~ N(0,1), exp stays well inside fp32 range.
Denominators (cross-partition sums of P') come from a pairwise bf16 DVE tree
plus one ones-vector matmul; 1/denom is computed with the fast approximate
reciprocal and broadcast across partitions by a stride-0 DMA.
QK^T / projections run in float32r (fp32 storage, 1 cycle/row at free>=256);
P' and V are bf16 (DVE 4x mode, PE fast-weight-load). Weight matrices are
transposed host-side, so no on-chip weight transposes are needed.
"""

import numpy as np

B = 8
C = 256
O = 256
N = 2304
NKT = 18  # key tiles of 128
BIG = [(0, 1024), (1024, 1024), (2048, 256)]  # query chunks
GROUPS = 8
EPS = 1e-5
SCALE = float(O) ** -0.5

_BUILD_CACHE = {}


def _subs(jw):
    return [(s, min(512, jw - s)) for s in range(0, jw, 512)]


def _build_nc():
    import concourse.mybir as mybir
    import concourse.tile as tile
    from concourse import bacc
    from concourse.bass import ts

    F32 = mybir.dt.float32
    F32R = mybir.dt.float32r
    BF16 = mybir.dt.bfloat16
    AF = mybir.ActivationFunctionType
    ALU = mybir.AluOpType
    AX = mybir.AxisListType

    nc = bacc.Bacc("TRN2", target_bir_lowering=False)

    f1_d = nc.dram_tensor("features1", [C, N], F32R, kind="ExternalInput")
    f2_d = nc.dram_tensor("features2", [C, N], F32R, kind="ExternalInput")
    # host-pre-transposed weights
    qwT_d = nc.dram_tensor("qwT", [C, O], F32R, kind="ExternalInput")
    kwT_d = nc.dram_tensor("kwT", [C, O], F32R, kind="ExternalInput")
    vwT_d = nc.dram_tensor("vwT", [C, O], F32R, kind="ExternalInput")
    owT_d = nc.dram_tensor("owT", [O, O], F32R, kind="ExternalInput")
    qb_d = nc.dram_tensor("q_b", [O], F32, kind="ExternalInput")
    kb_d = nc.dram_tensor("k_b", [O], F32, kind="ExternalInput")
    vbb_d = nc.dram_tensor("vb_bcast", [128, O], F32, kind="ExternalInput")
    ob_d = nc.dram_tensor("o_b", [O], F32, kind="ExternalInput")
    gnw_d = nc.dram_tensor("gn_w", [O], F32, kind="ExternalInput")
    gnb_d = nc.dram_tensor("gn_b", [O], F32, kind="ExternalInput")
    gsel_d = nc.dram_tensor("gsel", [128, 2 * GROUPS], F32, kind="ExternalInput")
    gselT_d = nc.dram_tensor("gselT", [GROUPS, 2 * 128], F32, kind="ExternalInput")
    onesb_d = nc.dram_tensor("ones_bf", [128, 1], BF16, kind="ExternalInput")
    onesr_d = nc.dram_tensor("ones_row_r", [1, 128], F32R, kind="ExternalInput")
    out_d = nc.dram_tensor("out", [O, N], F32, kind="ExternalOutput")

    with tile.TileContext(nc) as tc:
        with (
            tc.tile_pool(name="consts", bufs=1) as consts,
            tc.tile_pool(name="weights", bufs=1) as wpool,
            tc.tile_pool(name="acts", bufs=1) as apool,
        ):
            # ---- feature loads first: they gate the first matmuls.
            # Each dma_start already fans out over 16 SDMA engines, but all
            # DMAs of one issuing engine share a FIFO ring -- so spread the
            # four big loads across four engine rings. f2 first: K gates QK^T.
            f1sb = [apool.tile([128, N], F32R, name=f"f1sb{t}") for t in range(2)]
            f2sb = [apool.tile([128, N], F32R, name=f"f2sb{t}") for t in range(2)]
            feng = {
                (0, 0): nc.sync,
                (0, 1): nc.scalar,
                (1, 0): nc.gpsimd,
                (1, 1): nc.gpsimd,
            }
            for fi, (sb, dr) in enumerate(((f2sb, f2_d), (f1sb, f1_d))):
                for t in range(2):
                    eng = feng[(fi, t)]
                    for c0 in (0, 1152):
                        eng.dma_start(
                            out=sb[t][:, c0 : c0 + 1152],
                            in_=dr[ts(t, 128), c0 : c0 + 1152],
                        )

            # ---- constants ----
            zero_col = consts.tile([128, 1], F32, name="zero_col")
            nc.vector.memset(zero_col, 0.0)
            ones_bf = consts.tile([128, 1], BF16, name="ones_bf")
            nc.sync.dma_start(out=ones_bf, in_=onesb_d[:, :])
            ones_row = consts.tile([1, 128], F32R, name="ones_row")
            nc.sync.dma_start(out=ones_row, in_=onesr_d[:, :])
            vb_bc = consts.tile([128, O], F32, name="vb_bc")
            nc.sync.dma_start(out=vb_bc, in_=vbb_d[:, :])
            gsel = consts.tile([128, 2 * GROUPS], F32, name="gsel")
            nc.sync.dma_start(out=gsel, in_=gsel_d[:, :])
            gselT = consts.tile([GROUPS, 2 * 128], F32, name="gselT")
            nc.sync.dma_start(out=gselT, in_=gselT_d[:, :])

            qb_c, kb_c, ob_c, gnw_c, gnb_c = [], [], [], [], []
            for t in range(2):
                for lst, src, nm in (
                    (qb_c, qb_d, "qb"),
                    (kb_c, kb_d, "kb"),
                    (ob_c, ob_d, "ob"),
                    (gnw_c, gnw_d, "gnw"),
                    (gnb_c, gnb_d, "gnb"),
                ):
                    col = consts.tile([128, 1], F32, name=f"{nm}{t}")
                    nc.sync.dma_start(out=col, in_=src[ts(t, 128)].unsqueeze(1))
                    lst.append(col)

            # ---- persistent weights / activations ----
            qwT = [wpool.tile([128, O], F32R, name=f"qwT{t}") for t in range(2)]
            kwT = [wpool.tile([128, O], F32R, name=f"kwT{t}") for t in range(2)]
            vwT = [wpool.tile([128, O], F32R, name=f"vwT{t}") for t in range(2)]
            owT = [wpool.tile([128, O], F32R, name=f"owT{t}") for t in range(2)]
            for wt, wd in ((kwT, kwT_d), (vwT, vwT_d), (qwT, qwT_d), (owT, owT_d)):
                for t in range(2):
                    nc.sync.dma_start(out=wt[t], in_=wd[ts(t, 128), :])
            Q = [apool.tile([128, N], F32R, name=f"Q{t}") for t in range(2)]
            K = [apool.tile([128, N], F32R, name=f"K{t}") for t in range(2)]
            V = [apool.tile([128, O], BF16, name=f"V{k}") for k in range(NKT)]
            Z = [apool.tile([128, N], F32, name=f"Z{t}") for t in range(2)]
            # running GroupNorm stats per p-tile: col0 = sum, col1 = sumsq
            st_sums = [apool.tile([128, 2], F32, name=f"st{t}") for t in range(2)]
            for t in range(2):
                nc.vector.memset(st_sums[t], 0.0)

            # ---- phase 1: Q/K/V projections ----
            with (
                tc.tile_pool(name="fch", bufs=3) as fpool,
                tc.tile_pool(name="pps", bufs=4, space="PSUM") as pps,
            ):
                pchunks = ((0, 512), (512, 512), (1024, 512), (1536, 512), (2048, 256))
                for j0, jw in pchunks:
                    fa = fpool.tile([128, jw], F32R, tag="fa", name="fa")
                    fb = fpool.tile([128, jw], F32R, tag="fb", name="fb")
                    nc.sync.dma_start(out=fa, in_=f1_d[0:128, j0 : j0 + jw])
                    nc.sync.dma_start(out=fb, in_=f1_d[128:256, j0 : j0 + jw])
                    for t in range(2):
                        qp = pps.tile([128, jw], F32, tag="pp", name="qp")
                        nc.tensor.matmul(
                            qp, qwT[0][:, ts(t, 128)], fa, start=True, stop=False
                        )
                        nc.tensor.matmul(
                            qp, qwT[1][:, ts(t, 128)], fb, start=False, stop=True
                        )
                        nc.vector.tensor_scalar_add(
                            Q[t][:, j0 : j0 + jw], qp, qb_c[t]
                        )
                for j0, jw in pchunks:
                    fa = fpool.tile([128, jw], F32R, tag="fa", name="fa")
                    fb = fpool.tile([128, jw], F32R, tag="fb", name="fb")
                    nc.sync.dma_start(out=fa, in_=f2_d[0:128, j0 : j0 + jw])
                    nc.sync.dma_start(out=fb, in_=f2_d[128:256, j0 : j0 + jw])
                    for t in range(2):
                        kp = pps.tile([128, jw], F32, tag="pp", name="kp")
                        nc.tensor.matmul(
                            kp, kwT[0][:, ts(t, 128)], fa, start=True, stop=False
                        )
                        nc.tensor.matmul(
                            kp, kwT[1][:, ts(t, 128)], fb, start=False, stop=True
                        )
                        nc.vector.tensor_scalar_add(
                            K[t][:, j0 : j0 + jw], kp, kb_c[t]
                        )
                    for s in range(jw // 128):
                        nk = j0 // 128 + s
                        vp = pps.tile([128, O], F32, tag="pp", name="vp")
                        nc.tensor.matmul(
                            vp, fa[:, ts(s, 128)], vwT[0], start=True, stop=False
                        )
                        nc.tensor.matmul(
                            vp, fb[:, ts(s, 128)], vwT[1], start=False, stop=True
                        )
                        nc.vector.tensor_add(V[nk], vp, vb_bc)

            # ---- phase 2: attention + output projection, per query chunk ----
            with (
                tc.tile_pool(name="ppool", bufs=1) as ppool,
                tc.tile_pool(name="tpool", bufs=1) as tpool,
                tc.tile_pool(name="sbm", bufs=2) as sbm,
                tc.tile_pool(name="sps", bufs=2, space="PSUM") as sps,
                tc.tile_pool(name="ops", bufs=2, space="PSUM") as ops,
                tc.tile_pool(name="zps", bufs=1, space="PSUM") as zps,
                tc.tile_pool(name="dps", bufs=1, space="PSUM") as dps,
            ):
                for j0, jw in BIG:
                    subs = _subs(jw)
                    # scores (transposed) + exp, batched over the whole chunk
                    P = []
                    for nk in range(NKT):
                        sp = sps.tile([128, 1024], F32, tag="sp", name="sp")
                        for s0, sw in subs:
                            nc.tensor.matmul(
                                sp[:, s0 : s0 + sw],
                                K[0][:, ts(nk, 128)],
                                Q[0][:, j0 + s0 : j0 + s0 + sw],
                                start=True,
                                stop=False,
                            )
                            nc.tensor.matmul(
                                sp[:, s0 : s0 + sw],
                                K[1][:, ts(nk, 128)],
                                Q[1][:, j0 + s0 : j0 + s0 + sw],
                                start=False,
                                stop=True,
                            )
                        pt = ppool.tile([128, 1024], BF16, tag=f"p{nk}", name=f"pt{nk}")
                        nc.scalar.activation(
                            pt[:, :jw], sp[:, :jw], AF.Exp, bias=zero_col, scale=SCALE
                        )
                        P.append(pt)
                    # denominator: pairwise bf16 tree over the 18 P' tiles
                    tr = [
                        tpool.tile([128, 1024], BF16, tag=f"tr{i}", name=f"tr{i}")
                        for i in range(9)
                    ]
                    for i in range(9):
                        nc.vector.tensor_add(
                            tr[i][:, :jw], P[2 * i][:, :jw], P[2 * i + 1][:, :jw]
                        )
                    for i in range(4):
                        nc.vector.tensor_add(
                            tr[2 * i][:, :jw], tr[2 * i][:, :jw], tr[2 * i + 1][:, :jw]
                        )
                    nc.vector.tensor_add(tr[0][:, :jw], tr[0][:, :jw], tr[2][:, :jw])
                    nc.vector.tensor_add(tr[4][:, :jw], tr[4][:, :jw], tr[6][:, :jw])
                    nc.vector.tensor_add(tr[0][:, :jw], tr[0][:, :jw], tr[4][:, :jw])
                    nc.vector.tensor_add(tr[0][:, :jw], tr[0][:, :jw], tr[8][:, :jw])
                    for s0, sw in subs:
                        ssl = slice(s0, s0 + sw)
                        dn = dps.tile([1, 512], F32, tag="d", name="dn")
                        nc.tensor.matmul(
                            dn[:, :sw], ones_bf, tr[0][:, ssl], start=True, stop=True
                        )
                        dnr = sbm.tile([1, 512], F32R, tag="dnr", name="dnr")
                        nc.scalar.activation(dnr[:, :sw], dn[:, :sw], AF.Copy)
                        bc = dps.tile([128, 512], F32, tag="d", name="bc")
                        nc.tensor.matmul(
                            bc[:, :sw], ones_row, dnr[:, :sw], start=True, stop=True
                        )
                        bcs = sbm.tile([128, 512], F32, tag="bcs", name="bcs")
                        nc.vector.reciprocal_approx_fast(bcs[:, :sw], bc[:, :sw])
                        # att^T sub-chunk = (V^T P') * (1/denom) : [O, sw]
                        ATs = []
                        for o in range(2):
                            op = ops.tile([128, 512], F32, tag="op", name="op")
                            for nk in range(NKT):
                                nc.tensor.matmul(
                                    op[:, :sw],
                                    V[nk][:, ts(o, 128)],
                                    P[nk][:, ssl],
                                    start=(nk == 0),
                                    stop=(nk == NKT - 1),
                                )
                            at = sbm.tile([128, 512], F32R, tag=f"at{o}", name=f"at{o}")
                            nc.vector.tensor_mul(at[:, :sw], op[:, :sw], bcs[:, :sw])
                            ATs.append(at)
                        # output projection sub-chunk: Z[p, sw]
                        zsl = slice(j0 + s0, j0 + s0 + sw)
                        for p in range(2):
                            zp = zps.tile([128, 512], F32, tag="zp", name="zp")
                            nc.tensor.matmul(
                                zp[:, :sw],
                                owT[0][:, ts(p, 128)],
                                ATs[0][:, :sw],
                                start=True,
                                stop=False,
                            )
                            nc.tensor.matmul(
                                zp[:, :sw],
                                owT[1][:, ts(p, 128)],
                                ATs[1][:, :sw],
                                start=False,
                                stop=True,
                            )
                            # evacuate + bias; accum_out gives GN row-sums free
                            part = sbm.tile(
                                [128, 2], F32, tag=f"part{p}", name=f"part{p}"
                            )
                            nc.vector.tensor_scalar(
                                Z[p][:, zsl],
                                zp[:, :sw],
                                ob_c[p],
                                0.0,
                                op0=ALU.add,
                                op1=ALU.add,
                                accum_out=part[:, 0:1],
                            )
                            sqs = sbm.tile([128, 512], F32, tag="sqs", name="sqs")
                            nc.scalar.activation(
                                sqs[:, :sw],
                                Z[p][:, zsl],
                                AF.Square,
                                bias=zero_col,
                                accum_out=part[:, 1:2],
                            )
                            nc.vector.tensor_add(st_sums[p], st_sums[p], part)

            # ---- phase 3: GroupNorm finalization ----
            with (
                tc.tile_pool(name="gns", bufs=2) as gns,
                tc.tile_pool(name="gout", bufs=2) as gout,
                tc.tile_pool(name="gps", bufs=2, space="PSUM") as gps,
            ):
                gst = gps.tile([GROUPS, 2], F32, tag="gst", name="gst")
                nc.tensor.matmul(
                    gst, gsel[:, 0:GROUPS], st_sums[0], start=True, stop=False
                )
                nc.tensor.matmul(
                    gst,
                    gsel[:, GROUPS : 2 * GROUPS],
                    st_sums[1],
                    start=False,
                    stop=True,
                )
                # per-group mean / rstd on 8 partitions
                mv = gns.tile([GROUPS, 8], F32, tag="mv", name="mv")
                inv_cnt = 1.0 / (32.0 * N)
                nc.vector.tensor_scalar_mul(mv[:, 0:1], gst[:, 0:1], inv_cnt)  # mean
                nc.vector.tensor_scalar_mul(mv[:, 1:2], gst[:, 1:2], inv_cnt)  # E[x^2]
                nc.vector.tensor_mul(mv[:, 2:3], mv[:, 0:1], mv[:, 0:1])  # mean^2
                nc.vector.tensor_sub(mv[:, 3:4], mv[:, 1:2], mv[:, 2:3])  # var
                nc.vector.tensor_scalar_add(mv[:, 3:4], mv[:, 3:4], EPS)  # var+eps
                nc.scalar.activation(
                    mv[:, 4:5], mv[:, 3:4], AF.Sqrt, bias=zero_col[0:GROUPS, :]
                )
                nc.vector.reciprocal(mv[:, 5:6], mv[:, 4:5])  # y0 ~ rsqrt
                # one Newton step: y1 = y0 * (1.5 - 0.5*(var+eps)*y0^2)
                nc.vector.tensor_mul(mv[:, 6:7], mv[:, 5:6], mv[:, 5:6])  # y0^2
                nc.vector.tensor_mul(mv[:, 6:7], mv[:, 6:7], mv[:, 3:4])  # v*y0^2
                nc.vector.tensor_scalar(
                    mv[:, 6:7], mv[:, 6:7], -0.5, 1.5, op0=ALU.mult, op1=ALU.add
                )
                nc.vector.tensor_mul(mv[:, 7:8], mv[:, 5:6], mv[:, 6:7])  # rstd
                gm2 = gns.tile([GROUPS, 2], F32, tag="gm2", name="gm2")
                nc.vector.tensor_copy(gm2[:, 0:1], mv[:, 0:1])
                nc.vector.tensor_copy(gm2[:, 1:2], mv[:, 7:8])
                for p in range(2):
                    pst = gps.tile([128, 2], F32, tag="pst", name="pst")
                    nc.tensor.matmul(
                        pst, gselT[:, ts(p, 128)], gm2, start=True, stop=True
                    )
                    a_col = gns.tile([128, 1], F32, tag="a_col", name="a_col")
                    nc.vector.tensor_mul(a_col, pst[:, 1:2], gnw_c[p])
                    t_col = gns.tile([128, 1], F32, tag="t_col", name="t_col")
                    nc.vector.tensor_mul(t_col, pst[:, 0:1], a_col)
                    b_col = gns.tile([128, 1], F32, tag="b_col", name="b_col")
                    nc.vector.tensor_sub(b_col, gnb_c[p], t_col)
                    # scale+shift in column halves so output DMA overlaps compute
                    for h0 in (0, 1152):
                        outp = gout.tile([128, 1152], F32, tag="outp", name="outp")
                        nc.vector.tensor_scalar(
                            outp,
                            Z[p][:, h0 : h0 + 1152],
                            a_col,
                            b_col,
                            op0=ALU.mult,
                            op1=ALU.add,
                        )
                        nc.sync.dma_start(
                            out=out_d[ts(p, 128), h0 : h0 + 1152], in_=outp
                        )

    nc.finalize()
    return nc


def _get_nc():
    if "nc" not in _BUILD_CACHE:
        _BUILD_CACHE["nc"] = _build_nc()
    return _BUILD_CACHE["nc"]


def _make_in_maps(inputs):
    import ml_dtypes

    f1 = np.ascontiguousarray(
        np.asarray(inputs["features1"], dtype=np.float32).reshape(B, C, N)
    )
    f2 = np.ascontiguousarray(
        np.asarray(inputs["features2"], dtype=np.float32).reshape(B, C, N)
    )

    def g(k):
        return np.asarray(inputs[k], dtype=np.float32)

    gsel = np.zeros((128, 2 * GROUPS), np.float32)
    gselT = np.zeros((GROUPS, 2 * 128), np.float32)
    for t in range(2):
        for gl in range(4):
            grp = 4 * t + gl
            gsel[gl * 32 : (gl + 1) * 32, GROUPS * t + grp] = 1.0
            gselT[grp, 128 * t + gl * 32 : 128 * t + (gl + 1) * 32] = 1.0
    shared = {
        "qwT": np.ascontiguousarray(g("q_w").T),
        "kwT": np.ascontiguousarray(g("k_w").T),
        "vwT": np.ascontiguousarray(g("v_w").T),
        "owT": np.ascontiguousarray(g("o_w").T),
        "q_b": g("q_b"),
        "k_b": g("k_b"),
        "vb_bcast": np.ascontiguousarray(np.tile(g("v_b")[None, :], (128, 1))),
        "o_b": g("o_b"),
        "gn_w": g("gn_w"),
        "gn_b": g("gn_b"),
        "gsel": gsel,
        "gselT": gselT,
        "ones_bf": np.ones((128, 1), ml_dtypes.bfloat16),
        "ones_row_r": np.ones((1, 128), np.float32),
    }
    return [{"features1": f1[i], "features2": f2[i], **shared} for i in range(B)]


def run(inputs, trace=False):
    from concourse.bass_utils import run_bass_kernel_spmd

    nc = _get_nc()
    in_maps = _make_in_maps(inputs)
    res = run_bass_kernel_spmd(nc, in_maps, core_ids=list(range(B)), trace=trace)
    out = np.stack([np.asarray(res.results[i]["out"]) for i in range(B)])
    return out.reshape(B, O, 48, 48).astype(np.float32), res


def kernel(**inputs):
    out, _ = run(inputs, trace=False)
    return out


# revision 16
# speedup vs baseline: 1.3191x; 1.0333x over previous
"""Trainium2 Bass kernel for CrossAttentionFusion.

Reference computation (per batch b):
    Q = q_w @ f1 + q_b          (O, N)   f1 = features1[b] as (C, N)
    K = k_w @ f2 + k_b          (O, N)
    V = v_w @ f2 + v_b          -> used as (N, O)
    A = softmax(Q^T K / sqrt(O))  over keys          (N, N)
    att = A @ V                  (N, O)
    Z = o_w @ att^T + o_b        (O, N)
    out = GroupNorm(8 groups over O, spatial N)(Z) * gn_w + gn_b

Sharding: pure data-parallel, batch b -> NeuronCore b (B=8, 8 cores).

Layout trick: scores are computed transposed (S' = K^T Q in [nk, nq] tiles)
so the exp output P' feeds the A@V matmul directly (lhsT = V tile, rhs = P')
with zero on-chip transposes of the big attention matrix. Softmax needs no
max-subtraction: scores ~ N(0,1), exp stays well inside fp32 range.
Denominators (cross-partition sums of P') come from a pairwise bf16 DVE tree
plus one ones-vector matmul; 1/denom is computed with the fast approximate
reciprocal and broadcast across partitions by a stride-0 DMA.
QK^T / projections run in float32r (fp32 storage, 1 cycle/row at free>=256);
P' and V are bf16 (DVE 4x mode, PE fast-weight-load). Weight matrices are
transposed host-side, so no on-chip weight transposes are needed.
"""

import numpy as np

B = 8
C = 256
O = 256
N = 2304
NKT = 18  # key tiles of 128
BIG = [(0, 1024), (1024, 1024), (2048, 256)]  # query chunks
GROUPS = 8
EPS = 1e-5
SCALE = float(O) ** -0.5

_BUILD_CACHE = {}


def _subs(jw):
    return [(s, min(512, jw - s)) for s in range(0, jw, 512)]


def _build_nc():
    import concourse.mybir as mybir
    import concourse.tile as tile
    from concourse import bacc
    from concourse.bass import ts

    F32 = mybir.dt.float32
    F32R = mybir.dt.float32r
    BF16 = mybir.dt.bfloat16
    AF = mybir.ActivationFunctionType
    ALU = mybir.AluOpType
    AX = mybir.AxisListType

    nc = bacc.Bacc("TRN2", target_bir_lowering=False)

    f1_d = nc.dram_tensor("features1", [C, N], F32R, kind="ExternalInput")
    f2_d = nc.dram_tensor("features2", [C, N], F32R, kind="ExternalInput")
    # host-pre-transposed weights
    qwT_d = nc.dram_tensor("qwT", [C, O], F32R, kind="ExternalInput")
    kwT_d = nc.dram_tensor("kwT", [C, O], F32R, kind="ExternalInput")
    vwT_d = nc.dram_tensor("vwT", [C, O], F32R, kind="ExternalInput")
    owT_d = nc.dram_tensor("owT", [O, O], F32R, kind="ExternalInput")
    qb_d = nc.dram_tensor("q_b", [O], F32, kind="ExternalInput")
    kb_d = nc.dram_tensor("k_b", [O], F32, kind="ExternalInput")
    vbb_d = nc.dram_tensor("vb_bcast", [128, O], F32, kind="ExternalInput")
    ob_d = nc.dram_tensor("o_b", [O], F32, kind="ExternalInput")
    gnw_d = nc.dram_tensor("gn_w", [O], F32, kind="ExternalInput")
    gnb_d = nc.dram_tensor("gn_b", [O], F32, kind="ExternalInput")
    gsel_d = nc.dram_tensor("gsel", [128, 2 * GROUPS], F32, kind="ExternalInput")
    gselT_d = nc.dram_tensor("gselT", [GROUPS, 2 * 128], F32, kind="ExternalInput")
    onesb_d = nc.dram_tensor("ones_bf", [128, 1], BF16, kind="ExternalInput")
    onesr_d = nc.dram_tensor("ones_row_r", [1, 128], F32R, kind="ExternalInput")
    out_d = nc.dram_tensor("out", [O, N], F32, kind="ExternalOutput")

    with tile.TileContext(nc) as tc:
        with (
            tc.tile_pool(name="consts", bufs=1) as consts,
            tc.tile_pool(name="weights", bufs=1) as wpool,
            tc.tile_pool(name="acts", bufs=1) as apool,
        ):
            # ---- feature loads first: they gate the first matmuls.
            # Each dma_start already fans out over 16 SDMA engines, but all
            # DMAs of one issuing engine share a FIFO ring -- so spread the
            # four big loads across four engine rings. f2 first: K gates QK^T.
            f1sb = [apool.tile([128, N], F32R, name=f"f1sb{t}") for t in range(2)]
            f2sb = [apool.tile([128, N], F32R, name=f"f2sb{t}") for t in range(2)]
            # one FIFO ring (sync) in need-order: f2 pieces fully drain
            # before f1 pieces start pulling bandwidth
            for c0 in (0, 1152):
                for t in range(2):
                    nc.sync.dma_start(
                        out=f2sb[t][:, c0 : c0 + 1152],
                        in_=f2_d[ts(t, 128), c0 : c0 + 1152],
                    )
            for c0 in (0, 1152):
                for t in range(2):
                    nc.sync.dma_start(
                        out=f1sb[t][:, c0 : c0 + 1152],
                        in_=f1_d[ts(t, 128), c0 : c0 + 1152],
                    )

            # ---- constants ----
            zero_col = consts.tile([128, 1], F32, name="zero_col")
            nc.vector.memset(zero_col, 0.0)
            ones_bf = consts.tile([128, 1], BF16, name="ones_bf")
            nc.gpsimd.dma_start(out=ones_bf, in_=onesb_d[:, :])
            ones_row = consts.tile([1, 128], F32R, name="ones_row")
            nc.gpsimd.dma_start(out=ones_row, in_=onesr_d[:, :])
            vb_bc = consts.tile([128, O], F32, name="vb_bc")
            nc.gpsimd.dma_start(out=vb_bc, in_=vbb_d[:, :])
            gsel = consts.tile([128, 2 * GROUPS], F32, name="gsel")
            nc.gpsimd.dma_start(out=gsel, in_=gsel_d[:, :])
            gselT = consts.tile([GROUPS, 2 * 128], F32, name="gselT")
            nc.gpsimd.dma_start(out=gselT, in_=gselT_d[:, :])

            qb_c, kb_c, ob_c, gnw_c, gnb_c = [], [], [], [], []
            for t in range(2):
                for lst, src, nm in (
                    (qb_c, qb_d, "qb"),
                    (kb_c, kb_d, "kb"),
                    (ob_c, ob_d, "ob"),
                    (gnw_c, gnw_d, "gnw"),
                    (gnb_c, gnb_d, "gnb"),
                ):
                    col = consts.tile([128, 1], F32, name=f"{nm}{t}")
                    nc.gpsimd.dma_start(out=col, in_=src[ts(t, 128)].unsqueeze(1))
                    lst.append(col)

            # ---- persistent weights / activations ----
            qwT = [wpool.tile([128, O], F32R, name=f"qwT{t}") for t in range(2)]
            kwT = [wpool.tile([128, O], F32R, name=f"kwT{t}") for t in range(2)]
            vwT = [wpool.tile([128, O], F32R, name=f"vwT{t}") for t in range(2)]
            owT = [wpool.tile([128, O], F32R, name=f"owT{t}") for t in range(2)]
            for wt, wd in ((kwT, kwT_d), (vwT, vwT_d), (qwT, qwT_d), (owT, owT_d)):
                for t in range(2):
                    nc.scalar.dma_start(out=wt[t], in_=wd[ts(t, 128), :])
            Q = [apool.tile([128, N], F32R, name=f"Q{t}") for t in range(2)]
            K = [apool.tile([128, N], F32R, name=f"K{t}") for t in range(2)]
            V = [apool.tile([128, O], BF16, name=f"V{k}") for k in range(NKT)]
            Z = [apool.tile([128, N], F32, name=f"Z{t}") for t in range(2)]
            # running GroupNorm stats per p-tile: col0 = sum, col1 = sumsq
            st_sums = [apool.tile([128, 2], F32, name=f"st{t}") for t in range(2)]
            for t in range(2):
                nc.vector.memset(st_sums[t], 0.0)

            # ---- phase 1: Q/K/V projections ----
            with (
                tc.tile_pool(name="fch", bufs=3) as fpool,
                tc.tile_pool(name="pps", bufs=4, space="PSUM") as pps,
            ):
                pchunks = ((0, 512), (512, 512), (1024, 512), (1536, 512), (2048, 256))
                for j0, jw in pchunks:
                    fa = fpool.tile([128, jw], F32R, tag="fa", name="fa")
                    fb = fpool.tile([128, jw], F32R, tag="fb", name="fb")
                    nc.sync.dma_start(out=fa, in_=f1_d[0:128, j0 : j0 + jw])
                    nc.sync.dma_start(out=fb, in_=f1_d[128:256, j0 : j0 + jw])
                    for t in range(2):
                        qp = pps.tile([128, jw], F32, tag="pp", name="qp")
                        nc.tensor.matmul(
                            qp, qwT[0][:, ts(t, 128)], fa, start=True, stop=False
                        )
                        nc.tensor.matmul(
                            qp, qwT[1][:, ts(t, 128)], fb, start=False, stop=True
                        )
                        nc.vector.tensor_scalar_add(
                            Q[t][:, j0 : j0 + jw], qp, qb_c[t]
                        )
                for j0, jw in pchunks:
                    fa = fpool.tile([128, jw], F32R, tag="fa", name="fa")
                    fb = fpool.tile([128, jw], F32R, tag="fb", name="fb")
                    nc.sync.dma_start(out=fa, in_=f2_d[0:128, j0 : j0 + jw])
                    nc.sync.dma_start(out=fb, in_=f2_d[128:256, j0 : j0 + jw])
                    for t in range(2):
                        kp = pps.tile([128, jw], F32, tag="pp", name="kp")
                        nc.tensor.matmul(
                            kp, kwT[0][:, ts(t, 128)], fa, start=True, stop=False
                        )
                        nc.tensor.matmul(
                            kp, kwT[1][:, ts(t, 128)], fb, start=False, stop=True
                        )
                        nc.vector.tensor_scalar_add(
                            K[t][:, j0 : j0 + jw], kp, kb_c[t]
                        )
                    for s in range(jw // 128):
                        nk = j0 // 128 + s
                        vp = pps.tile([128, O], F32, tag="pp", name="vp")
                        nc.tensor.matmul(
                            vp, fa[:, ts(s, 128)], vwT[0], start=True, stop=False
                        )
                        nc.tensor.matmul(
                            vp, fb[:, ts(s, 128)], vwT[1], start=False, stop=True
                        )
                        nc.vector.tensor_add(V[nk], vp, vb_bc)

            # ---- phase 2: attention + output projection, per query chunk ----
            with (
                tc.tile_pool(name="ppool", bufs=1) as ppool,
                tc.tile_pool(name="tpool", bufs=1) as tpool,
                tc.tile_pool(name="sbm", bufs=2) as sbm,
                tc.tile_pool(name="sps", bufs=2, space="PSUM") as sps,
                tc.tile_pool(name="ops", bufs=2, space="PSUM") as ops,
                tc.tile_pool(name="zps", bufs=1, space="PSUM") as zps,
                tc.tile_pool(name="dps", bufs=1, space="PSUM") as dps,
            ):
                for j0, jw in BIG:
                    subs = _subs(jw)
                    # scores (transposed) + exp, batched over the whole chunk
                    P = []
                    for nk in range(NKT):
                        sp = sps.tile([128, 1024], F32, tag="sp", name="sp")
                        for s0, sw in subs:
                            nc.tensor.matmul(
                                sp[:, s0 : s0 + sw],
                                K[0][:, ts(nk, 128)],
                                Q[0][:, j0 + s0 : j0 + s0 + sw],
                                start=True,
                                stop=False,
                            )
                            nc.tensor.matmul(
                                sp[:, s0 : s0 + sw],
                                K[1][:, ts(nk, 128)],
                                Q[1][:, j0 + s0 : j0 + s0 + sw],
                                start=False,
                                stop=True,
                            )
                        pt = ppool.tile([128, 1024], BF16, tag=f"p{nk}", name=f"pt{nk}")
                        nc.scalar.activation(
                            pt[:, :jw], sp[:, :jw], AF.Exp, bias=zero_col, scale=SCALE
                        )
                        P.append(pt)
                    # denominator: pairwise bf16 tree over the 18 P' tiles
                    tr = [
                        tpool.tile([128, 1024], BF16, tag=f"tr{i}", name=f"tr{i}")
                        for i in range(9)
                    ]
                    for i in range(9):
                        nc.vector.tensor_add(
                            tr[i][:, :jw], P[2 * i][:, :jw], P[2 * i + 1][:, :jw]
                        )
                    for i in range(4):
                        nc.vector.tensor_add(
                            tr[2 * i][:, :jw], tr[2 * i][:, :jw], tr[2 * i + 1][:, :jw]
                        )
                    nc.vector.tensor_add(tr[0][:, :jw], tr[0][:, :jw], tr[2][:, :jw])
                    nc.vector.tensor_add(tr[4][:, :jw], tr[4][:, :jw], tr[6][:, :jw])
                    nc.vector.tensor_add(tr[0][:, :jw], tr[0][:, :jw], tr[4][:, :jw])
                    nc.vector.tensor_add(tr[0][:, :jw], tr[0][:, :jw], tr[8][:, :jw])
                    for s0, sw in subs:
                        ssl = slice(s0, s0 + sw)
                        dn = dps.tile([1, 512], F32, tag="d", name="dn")
                        nc.tensor.matmul(
                            dn[:, :sw], ones_bf, tr[0][:, ssl], start=True, stop=True
                        )
                        dnr = sbm.tile([1, 512], F32R, tag="dnr", name="dnr")
                        nc.scalar.activation(dnr[:, :sw], dn[:, :sw], AF.Copy)
                        bc = dps.tile([128, 512], F32, tag="d", name="bc")
                        nc.tensor.matmul(
                            bc[:, :sw], ones_row, dnr[:, :sw], start=True, stop=True
                        )
                        bcs = sbm.tile([128, 512], F32, tag="bcs", name="bcs")
                        nc.vector.reciprocal_approx_fast(bcs[:, :sw], bc[:, :sw])
                        # att^T sub-chunk = (V^T P') * (1/denom) : [O, sw]
                        ATs = []
                        for o in range(2):
                            op = ops.tile([128, 512], F32, tag="op", name="op")
                            for nk in range(NKT):
                                nc.tensor.matmul(
                                    op[:, :sw],
                                    V[nk][:, ts(o, 128)],
                                    P[nk][:, ssl],
                                    start=(nk == 0),
                                    stop=(nk == NKT - 1),
                                )
                            at = sbm.tile([128, 512], F32R, tag=f"at{o}", name=f"at{o}")
                            nc.vector.tensor_mul(at[:, :sw], op[:, :sw], bcs[:, :sw])
                            ATs.append(at)
                        # output projection sub-chunk: Z[p, sw]
                        zsl = slice(j0 + s0, j0 + s0 + sw)
                        for p in range(2):
                            zp = zps.tile([128, 512], F32, tag="zp", name="zp")
                            nc.tensor.matmul(
                                zp[:, :sw],
                                owT[0][:, ts(p, 128)],
                                ATs[0][:, :sw],
                                start=True,
                                stop=False,
                            )
                            nc.tensor.matmul(
                                zp[:, :sw],
                                owT[1][:, ts(p, 128)],
                                ATs[1][:, :sw],
                                start=False,
                                stop=True,
                            )
                            # evacuate + bias; accum_out gives GN row-sums free
                            part = sbm.tile(
                                [128, 2], F32, tag=f"part{p}", name=f"part{p}"
                            )
                            nc.vector.tensor_scalar(
                                Z[p][:, zsl],
                                zp[:, :sw],
                                ob_c[p],
                                0.0,
                                op0=ALU.add,
                                op1=ALU.add,
                                accum_out=part[:, 0:1],
                            )
                            sqs = sbm.tile([128, 512], F32, tag="sqs", name="sqs")
                            nc.scalar.activation(
                                sqs[:, :sw],
                                Z[p][:, zsl],
                                AF.Square,
                                bias=zero_col,
                                accum_out=part[:, 1:2],
                            )
                            nc.vector.tensor_add(st_sums[p], st_sums[p], part)

            # ---- phase 3: GroupNorm finalization ----
            with (
                tc.tile_pool(name="gns", bufs=2) as gns,
                tc.tile_pool(name="gout", bufs=2) as gout,
                tc.tile_pool(name="gps", bufs=2, space="PSUM") as gps,
            ):
                gst = gps.tile([GROUPS, 2], F32, tag="gst", name="gst")
                nc.tensor.matmul(
                    gst, gsel[:, 0:GROUPS], st_sums[0], start=True, stop=False
                )
                nc.tensor.matmul(
                    gst,
                    gsel[:, GROUPS : 2 * GROUPS],
                    st_sums[1],
                    start=False,
                    stop=True,
                )
                # per-group mean / rstd on 8 partitions
                mv = gns.tile([GROUPS, 8], F32, tag="mv", name="mv")
                inv_cnt = 1.0 / (32.0 * N)
                nc.vector.tensor_scalar_mul(mv[:, 0:1], gst[:, 0:1], inv_cnt)  # mean
                nc.vector.tensor_scalar_mul(mv[:, 1:2], gst[:, 1:2], inv_cnt)  # E[x^2]
                nc.vector.tensor_mul(mv[:, 2:3], mv[:, 0:1], mv[:, 0:1])  # mean^2
                nc.vector.tensor_sub(mv[:, 3:4], mv[:, 1:2], mv[:, 2:3])  # var
                nc.vector.tensor_scalar_add(mv[:, 3:4], mv[:, 3:4], EPS)  # var+eps
                nc.scalar.activation(
                    mv[:, 4:5], mv[:, 3:4], AF.Sqrt, bias=zero_col[0:GROUPS, :]
                )
                nc.vector.reciprocal(mv[:, 5:6], mv[:, 4:5])  # y0 ~ rsqrt
                # one Newton step: y1 = y0 * (1.5 - 0.5*(var+eps)*y0^2)
                nc.vector.tensor_mul(mv[:, 6:7], mv[:, 5:6], mv[:, 5:6])  # y0^2
                nc.vector.tensor_mul(mv[:, 6:7], mv[:, 6:7], mv[:, 3:4])  # v*y0^2
                nc.vector.tensor_scalar(
                    mv[:, 6:7], mv[:, 6:7], -0.5, 1.5, op0=ALU.mult, op1=ALU.add
                )
                nc.vector.tensor_mul(mv[:, 7:8], mv[:, 5:6], mv[:, 6:7])  # rstd
                gm2 = gns.tile([GROUPS, 2], F32, tag="gm2", name="gm2")
                nc.vector.tensor_copy(gm2[:, 0:1], mv[:, 0:1])
                nc.vector.tensor_copy(gm2[:, 1:2], mv[:, 7:8])
                for p in range(2):
                    pst = gps.tile([128, 2], F32, tag="pst", name="pst")
                    nc.tensor.matmul(
                        pst, gselT[:, ts(p, 128)], gm2, start=True, stop=True
                    )
                    a_col = gns.tile([128, 1], F32, tag="a_col", name="a_col")
                    nc.vector.tensor_mul(a_col, pst[:, 1:2], gnw_c[p])
                    t_col = gns.tile([128, 1], F32, tag="t_col", name="t_col")
                    nc.vector.tensor_mul(t_col, pst[:, 0:1], a_col)
                    b_col = gns.tile([128, 1], F32, tag="b_col", name="b_col")
                    nc.vector.tensor_sub(b_col, gnb_c[p], t_col)
                    # scale+shift in column halves so output DMA overlaps compute
                    for h0 in (0, 1152):
                        outp = gout.tile([128, 1152], F32, tag="outp", name="outp")
                        nc.vector.tensor_scalar(
                            outp,
                            Z[p][:, h0 : h0 + 1152],
                            a_col,
                            b_col,
                            op0=ALU.mult,
                            op1=ALU.add,
                        )
                        nc.sync.dma_start(
                            out=out_d[ts(p, 128), h0 : h0 + 1152], in_=outp
                        )

    nc.finalize()
    return nc


def _get_nc():
    if "nc" not in _BUILD_CACHE:
        _BUILD_CACHE["nc"] = _build_nc()
    return _BUILD_CACHE["nc"]


def _make_in_maps(inputs):
    import ml_dtypes

    f1 = np.ascontiguousarray(
        np.asarray(inputs["features1"], dtype=np.float32).reshape(B, C, N)
    )
    f2 = np.ascontiguousarray(
        np.asarray(inputs["features2"], dtype=np.float32).reshape(B, C, N)
    )

    def g(k):
        return np.asarray(inputs[k], dtype=np.float32)

    gsel = np.zeros((128, 2 * GROUPS), np.float32)
    gselT = np.zeros((GROUPS, 2 * 128), np.float32)
    for t in range(2):
        for gl in range(4):
            grp = 4 * t + gl
            gsel[gl * 32 : (gl + 1) * 32, GROUPS * t + grp] = 1.0
            gselT[grp, 128 * t + gl * 32 : 128 * t + (gl + 1) * 32] = 1.0
    shared = {
        "qwT": np.ascontiguousarray(g("q_w").T),
        "kwT": np.ascontiguousarray(g("k_w").T),
        "vwT": np.ascontiguousarray(g("v_w").T),
        "owT": np.ascontiguousarray(g("o_w").T),
        "q_b": g("q_b"),
        "k_b": g("k_b"),
        "vb_bcast": np.ascontiguousarray(np.tile(g("v_b")[None, :], (128, 1))),
        "o_b": g("o_b"),
        "gn_w": g("gn_w"),
        "gn_b": g("gn_b"),
        "gsel": gsel,
        "gselT": gselT,
        "ones_bf": np.ones((128, 1), ml_dtypes.bfloat16),
        "ones_row_r": np.ones((1, 128), np.float32),
    }
    return [{"features1": f1[i], "features2": f2[i], **shared} for i in range(B)]


def run(inputs, trace=False):
    from concourse.bass_utils import run_bass_kernel_spmd

    nc = _get_nc()
    in_maps = _make_in_maps(inputs)
    res = run_bass_kernel_spmd(nc, in_maps, core_ids=list(range(B)), trace=trace)
    out = np.stack([np.asarray(res.results[i]["out"]) for i in range(B)])
    return out.reshape(B, O, 48, 48).astype(np.float32), res


def kernel(**inputs):
    out, _ = run(inputs, trace=False)
    return out


# revision 18
# speedup vs baseline: 1.4544x; 1.1026x over previous
"""Trainium2 Bass kernel for CrossAttentionFusion.

Reference computation (per batch b):
    Q = q_w @ f1 + q_b          (O, N)   f1 = features1[b] as (C, N)
    K = k_w @ f2 + k_b          (O, N)
    V = v_w @ f2 + v_b          -> used as (N, O)
    A = softmax(Q^T K / sqrt(O))  over keys          (N, N)
    att = A @ V                  (N, O)
    Z = o_w @ att^T + o_b        (O, N)
    out = GroupNorm(8 groups over O, spatial N)(Z) * gn_w + gn_b

Sharding: pure data-parallel, batch b -> NeuronCore b (B=8, 8 cores).

Layout trick: scores are computed transposed (S' = K^T Q in [nk, nq] tiles)
so the exp output P' feeds the A@V matmul directly (lhsT = V tile, rhs = P')
with zero on-chip transposes of the big attention matrix. Softmax needs no
max-subtraction: scores ~ N(0,1), exp stays well inside fp32 range.
Denominators (cross-partition sums of P') come from a pairwise bf16 DVE tree
plus one ones-vector matmul; 1/denom is computed with the fast approximate
reciprocal and broadcast across partitions by a stride-0 DMA.
QK^T / projections run in float32r (fp32 storage, 1 cycle/row at free>=256);
P' and V are bf16 (DVE 4x mode, PE fast-weight-load). Weight matrices are
transposed host-side, so no on-chip weight transposes are needed.
"""

import numpy as np

B = 8
C = 256
O = 256
N = 2304
NKT = 18  # key tiles of 128
BIG = [(0, 1024), (1024, 1024), (2048, 256)]  # query chunks
GROUPS = 8
EPS = 1e-5
SCALE = float(O) ** -0.5

_BUILD_CACHE = {}


def _subs(jw):
    return [(s, min(512, jw - s)) for s in range(0, jw, 512)]


def _build_nc():
    import concourse.mybir as mybir
    import concourse.tile as tile
    from concourse import bacc
    from concourse.bass import ts

    F32 = mybir.dt.float32
    F32R = mybir.dt.float32r
    BF16 = mybir.dt.bfloat16
    AF = mybir.ActivationFunctionType
    ALU = mybir.AluOpType
    AX = mybir.AxisListType

    nc = bacc.Bacc("TRN2", target_bir_lowering=False)

    f1_d = nc.dram_tensor("features1", [C, N], BF16, kind="ExternalInput")
    f2_d = nc.dram_tensor("features2", [C, N], BF16, kind="ExternalInput")
    # host-pre-transposed weights
    qwT_d = nc.dram_tensor("qwT", [C, O], BF16, kind="ExternalInput")
    kwT_d = nc.dram_tensor("kwT", [C, O], BF16, kind="ExternalInput")
    vwT_d = nc.dram_tensor("vwT", [C, O], BF16, kind="ExternalInput")
    owT_d = nc.dram_tensor("owT", [O, O], F32R, kind="ExternalInput")
    qb_d = nc.dram_tensor("q_b", [O], F32, kind="ExternalInput")
    kb_d = nc.dram_tensor("k_b", [O], F32, kind="ExternalInput")
    vbb_d = nc.dram_tensor("vb_bcast", [128, O], F32, kind="ExternalInput")
    ob_d = nc.dram_tensor("o_b", [O], F32, kind="ExternalInput")
    gnw_d = nc.dram_tensor("gn_w", [O], F32, kind="ExternalInput")
    gnb_d = nc.dram_tensor("gn_b", [O], F32, kind="ExternalInput")
    gsel_d = nc.dram_tensor("gsel", [128, 2 * GROUPS], F32, kind="ExternalInput")
    gselT_d = nc.dram_tensor("gselT", [GROUPS, 2 * 128], F32, kind="ExternalInput")
    onesb_d = nc.dram_tensor("ones_bf", [128, 1], BF16, kind="ExternalInput")
    onesr_d = nc.dram_tensor("ones_row_r", [1, 128], F32R, kind="ExternalInput")
    out_d = nc.dram_tensor("out", [O, N], F32, kind="ExternalOutput")

    with tile.TileContext(nc) as tc:
        with (
            tc.tile_pool(name="consts", bufs=1) as consts,
            tc.tile_pool(name="weights", bufs=1) as wpool,
            tc.tile_pool(name="acts", bufs=1) as apool,
        ):
            # ---- feature loads first: they gate the first matmuls.
            # Each dma_start already fans out over 16 SDMA engines, but all
            # DMAs of one issuing engine share a FIFO ring -- so spread the
            # four big loads across four engine rings. f2 first: K gates QK^T.
            f1sb = [apool.tile([128, N], BF16, name=f"f1sb{t}") for t in range(2)]
            f2sb = [apool.tile([128, N], BF16, name=f"f2sb{t}") for t in range(2)]
            # one FIFO ring (sync) in need-order: f2 pieces fully drain
            # before f1 pieces start pulling bandwidth
            for c0 in (0, 1152):
                for t in range(2):
                    nc.sync.dma_start(
                        out=f2sb[t][:, c0 : c0 + 1152],
                        in_=f2_d[ts(t, 128), c0 : c0 + 1152],
                    )
            for c0 in (0, 1152):
                for t in range(2):
                    nc.sync.dma_start(
                        out=f1sb[t][:, c0 : c0 + 1152],
                        in_=f1_d[ts(t, 128), c0 : c0 + 1152],
                    )

            # ---- constants ----
            zero_col = consts.tile([128, 1], F32, name="zero_col")
            nc.vector.memset(zero_col, 0.0)
            ones_bf = consts.tile([128, 1], BF16, name="ones_bf")
            nc.gpsimd.dma_start(out=ones_bf, in_=onesb_d[:, :])
            ones_row = consts.tile([1, 128], F32R, name="ones_row")
            nc.gpsimd.dma_start(out=ones_row, in_=onesr_d[:, :])
            vb_bc = consts.tile([128, O], F32, name="vb_bc")
            nc.gpsimd.dma_start(out=vb_bc, in_=vbb_d[:, :])
            gsel = consts.tile([128, 2 * GROUPS], F32, name="gsel")
            nc.gpsimd.dma_start(out=gsel, in_=gsel_d[:, :])
            gselT = consts.tile([GROUPS, 2 * 128], F32, name="gselT")
            nc.gpsimd.dma_start(out=gselT, in_=gselT_d[:, :])

            qb_c, kb_c, ob_c, gnw_c, gnb_c = [], [], [], [], []
            for t in range(2):
                for lst, src, nm in (
                    (qb_c, qb_d, "qb"),
                    (kb_c, kb_d, "kb"),
                    (ob_c, ob_d, "ob"),
                    (gnw_c, gnw_d, "gnw"),
                    (gnb_c, gnb_d, "gnb"),
                ):
                    col = consts.tile([128, 1], F32, name=f"{nm}{t}")
                    nc.gpsimd.dma_start(out=col, in_=src[ts(t, 128)].unsqueeze(1))
                    lst.append(col)

            # ---- persistent weights / activations ----
            qwT = [wpool.tile([128, O], BF16, name=f"qwT{t}") for t in range(2)]
            kwT = [wpool.tile([128, O], BF16, name=f"kwT{t}") for t in range(2)]
            vwT = [wpool.tile([128, O], BF16, name=f"vwT{t}") for t in range(2)]
            owT = [wpool.tile([128, O], F32R, name=f"owT{t}") for t in range(2)]
            for wt, wd in ((kwT, kwT_d), (vwT, vwT_d), (qwT, qwT_d), (owT, owT_d)):
                for t in range(2):
                    nc.scalar.dma_start(out=wt[t], in_=wd[ts(t, 128), :])
            Q = [apool.tile([128, N], BF16, name=f"Q{t}") for t in range(2)]
            K = [apool.tile([128, N], BF16, name=f"K{t}") for t in range(2)]
            V = [apool.tile([128, O], BF16, name=f"V{k}") for k in range(NKT)]
            Z = [apool.tile([128, N], F32, name=f"Z{t}") for t in range(2)]
            # running GroupNorm stats per p-tile: col0 = sum, col1 = sumsq
            st_sums = [apool.tile([128, 2], F32, name=f"st{t}") for t in range(2)]
            for t in range(2):
                nc.vector.memset(st_sums[t], 0.0)

            # ---- phase 1: Q/K/V projections (K/V first: they gate QK^T) ----
            with tc.tile_pool(name="pps", bufs=4, space="PSUM") as pps:
                pchunks = ((0, 512), (512, 512), (1024, 512), (1536, 512), (2048, 256))
                for j0, jw in pchunks:
                    jsl = slice(j0, j0 + jw)
                    for t in range(2):
                        kp = pps.tile([128, jw], F32, tag="pp", name="kp")
                        nc.tensor.matmul(
                            kp, kwT[0][:, ts(t, 128)], f2sb[0][:, jsl],
                            start=True, stop=False,
                        )
                        nc.tensor.matmul(
                            kp, kwT[1][:, ts(t, 128)], f2sb[1][:, jsl],
                            start=False, stop=True,
                        )
                        nc.vector.tensor_scalar_add(K[t][:, jsl], kp, kb_c[t])
                    for s in range(jw // 128):
                        nk = j0 // 128 + s
                        vp = pps.tile([128, O], F32, tag="pp", name="vp")
                        nc.tensor.matmul(
                            vp, f2sb[0][:, ts(nk, 128)], vwT[0], start=True, stop=False
                        )
                        nc.tensor.matmul(
                            vp, f2sb[1][:, ts(nk, 128)], vwT[1], start=False, stop=True
                        )
                        nc.vector.tensor_add(V[nk], vp, vb_bc)
                for j0, jw in pchunks:
                    jsl = slice(j0, j0 + jw)
                    for t in range(2):
                        qp = pps.tile([128, jw], F32, tag="pp", name="qp")
                        nc.tensor.matmul(
                            qp, qwT[0][:, ts(t, 128)], f1sb[0][:, jsl],
                            start=True, stop=False,
                        )
                        nc.tensor.matmul(
                            qp, qwT[1][:, ts(t, 128)], f1sb[1][:, jsl],
                            start=False, stop=True,
                        )
                        nc.vector.tensor_scalar_add(Q[t][:, jsl], qp, qb_c[t])

            # ---- phase 2: attention + output projection, per query chunk ----
            with (
                tc.tile_pool(name="ppool", bufs=1) as ppool,
                tc.tile_pool(name="tpool", bufs=1) as tpool,
                tc.tile_pool(name="sbm", bufs=2) as sbm,
                tc.tile_pool(name="sps", bufs=2, space="PSUM") as sps,
                tc.tile_pool(name="ops", bufs=2, space="PSUM") as ops,
                tc.tile_pool(name="zps", bufs=1, space="PSUM") as zps,
                tc.tile_pool(name="dps", bufs=1, space="PSUM") as dps,
            ):
                for j0, jw in BIG:
                    subs = _subs(jw)
                    # scores (transposed) + exp, batched over the whole chunk
                    P = []
                    for nk in range(NKT):
                        sp = sps.tile([128, 1024], F32, tag="sp", name="sp")
                        for s0, sw in subs:
                            nc.tensor.matmul(
                                sp[:, s0 : s0 + sw],
                                K[0][:, ts(nk, 128)],
                                Q[0][:, j0 + s0 : j0 + s0 + sw],
                                start=True,
                                stop=False,
                            )
                            nc.tensor.matmul(
                                sp[:, s0 : s0 + sw],
                                K[1][:, ts(nk, 128)],
                                Q[1][:, j0 + s0 : j0 + s0 + sw],
                                start=False,
                                stop=True,
                            )
                        pt = ppool.tile([128, 1024], BF16, tag=f"p{nk}", name=f"pt{nk}")
                        nc.scalar.activation(
                            pt[:, :jw], sp[:, :jw], AF.Exp, bias=zero_col, scale=SCALE
                        )
                        P.append(pt)
                    # denominator: pairwise bf16 tree over the 18 P' tiles
                    tr = [
                        tpool.tile([128, 1024], BF16, tag=f"tr{i}", name=f"tr{i}")
                        for i in range(9)
                    ]
                    for i in range(9):
                        nc.vector.tensor_add(
                            tr[i][:, :jw], P[2 * i][:, :jw], P[2 * i + 1][:, :jw]
                        )
                    for i in range(4):
                        nc.vector.tensor_add(
                            tr[2 * i][:, :jw], tr[2 * i][:, :jw], tr[2 * i + 1][:, :jw]
                        )
                    nc.vector.tensor_add(tr[0][:, :jw], tr[0][:, :jw], tr[2][:, :jw])
                    nc.vector.tensor_add(tr[4][:, :jw], tr[4][:, :jw], tr[6][:, :jw])
                    nc.vector.tensor_add(tr[0][:, :jw], tr[0][:, :jw], tr[4][:, :jw])
                    nc.vector.tensor_add(tr[0][:, :jw], tr[0][:, :jw], tr[8][:, :jw])
                    for s0, sw in subs:
                        ssl = slice(s0, s0 + sw)
                        dn = dps.tile([1, 512], F32, tag="d", name="dn")
                        nc.tensor.matmul(
                            dn[:, :sw], ones_bf, tr[0][:, ssl], start=True, stop=True
                        )
                        dnr = sbm.tile([1, 512], F32R, tag="dnr", name="dnr")
                        nc.scalar.activation(dnr[:, :sw], dn[:, :sw], AF.Copy)
                        bc = dps.tile([128, 512], F32, tag="d", name="bc")
                        nc.tensor.matmul(
                            bc[:, :sw], ones_row, dnr[:, :sw], start=True, stop=True
                        )
                        bcs = sbm.tile([128, 512], F32, tag="bcs", name="bcs")
                        nc.vector.reciprocal_approx_fast(bcs[:, :sw], bc[:, :sw])
                        # att^T sub-chunk = (V^T P') * (1/denom) : [O, sw]
                        ATs = []
                        for o in range(2):
                            op = ops.tile([128, 512], F32, tag="op", name="op")
                            for nk in range(NKT):
                                nc.tensor.matmul(
                                    op[:, :sw],
                                    V[nk][:, ts(o, 128)],
                                    P[nk][:, ssl],
                                    start=(nk == 0),
                                    stop=(nk == NKT - 1),
                                )
                            at = sbm.tile([128, 512], F32R, tag=f"at{o}", name=f"at{o}")
                            nc.vector.tensor_mul(at[:, :sw], op[:, :sw], bcs[:, :sw])
                            ATs.append(at)
                        # output projection sub-chunk: Z[p, sw]
                        zsl = slice(j0 + s0, j0 + s0 + sw)
                        for p in range(2):
                            zp = zps.tile([128, 512], F32, tag="zp", name="zp")
                            nc.tensor.matmul(
                                zp[:, :sw],
                                owT[0][:, ts(p, 128)],
                                ATs[0][:, :sw],
                                start=True,
                                stop=False,
                            )
                            nc.tensor.matmul(
                                zp[:, :sw],
                                owT[1][:, ts(p, 128)],
                                ATs[1][:, :sw],
                                start=False,
                                stop=True,
                            )
                            # evacuate + bias; accum_out gives GN row-sums free
                            part = sbm.tile(
                                [128, 2], F32, tag=f"part{p}", name=f"part{p}"
                            )
                            nc.vector.tensor_scalar(
                                Z[p][:, zsl],
                                zp[:, :sw],
                                ob_c[p],
                                0.0,
                                op0=ALU.add,
                                op1=ALU.add,
                                accum_out=part[:, 0:1],
                            )
                            sqs = sbm.tile([128, 512], F32, tag="sqs", name="sqs")
                            nc.scalar.activation(
                                sqs[:, :sw],
                                Z[p][:, zsl],
                                AF.Square,
                                bias=zero_col,
                                accum_out=part[:, 1:2],
                            )
                            nc.vector.tensor_add(st_sums[p], st_sums[p], part)

            # ---- phase 3: GroupNorm finalization ----
            with (
                tc.tile_pool(name="gns", bufs=2) as gns,
                tc.tile_pool(name="gout", bufs=2) as gout,
                tc.tile_pool(name="gps", bufs=2, space="PSUM") as gps,
            ):
                gst = gps.tile([GROUPS, 2], F32, tag="gst", name="gst")
                nc.tensor.matmul(
                    gst, gsel[:, 0:GROUPS], st_sums[0], start=True, stop=False
                )
                nc.tensor.matmul(
                    gst,
                    gsel[:, GROUPS : 2 * GROUPS],
                    st_sums[1],
                    start=False,
                    stop=True,
                )
                # per-group mean / rstd on 8 partitions
                mv = gns.tile([GROUPS, 8], F32, tag="mv", name="mv")
                inv_cnt = 1.0 / (32.0 * N)
                nc.vector.tensor_scalar_mul(mv[:, 0:1], gst[:, 0:1], inv_cnt)  # mean
                nc.vector.tensor_scalar_mul(mv[:, 1:2], gst[:, 1:2], inv_cnt)  # E[x^2]
                nc.vector.tensor_mul(mv[:, 2:3], mv[:, 0:1], mv[:, 0:1])  # mean^2
                nc.vector.tensor_sub(mv[:, 3:4], mv[:, 1:2], mv[:, 2:3])  # var
                nc.vector.tensor_scalar_add(mv[:, 3:4], mv[:, 3:4], EPS)  # var+eps
                nc.scalar.activation(
                    mv[:, 4:5], mv[:, 3:4], AF.Sqrt, bias=zero_col[0:GROUPS, :]
                )
                nc.vector.reciprocal(mv[:, 5:6], mv[:, 4:5])  # y0 ~ rsqrt
                # one Newton step: y1 = y0 * (1.5 - 0.5*(var+eps)*y0^2)
                nc.vector.tensor_mul(mv[:, 6:7], mv[:, 5:6], mv[:, 5:6])  # y0^2
                nc.vector.tensor_mul(mv[:, 6:7], mv[:, 6:7], mv[:, 3:4])  # v*y0^2
                nc.vector.tensor_scalar(
                    mv[:, 6:7], mv[:, 6:7], -0.5, 1.5, op0=ALU.mult, op1=ALU.add
                )
                nc.vector.tensor_mul(mv[:, 7:8], mv[:, 5:6], mv[:, 6:7])  # rstd
                gm2 = gns.tile([GROUPS, 2], F32, tag="gm2", name="gm2")
                nc.vector.tensor_copy(gm2[:, 0:1], mv[:, 0:1])
                nc.vector.tensor_copy(gm2[:, 1:2], mv[:, 7:8])
                for p in range(2):
                    pst = gps.tile([128, 2], F32, tag="pst", name="pst")
                    nc.tensor.matmul(
                        pst, gselT[:, ts(p, 128)], gm2, start=True, stop=True
                    )
                    a_col = gns.tile([128, 1], F32, tag="a_col", name="a_col")
                    nc.vector.tensor_mul(a_col, pst[:, 1:2], gnw_c[p])
                    t_col = gns.tile([128, 1], F32, tag="t_col", name="t_col")
                    nc.vector.tensor_mul(t_col, pst[:, 0:1], a_col)
                    b_col = gns.tile([128, 1], F32, tag="b_col", name="b_col")
                    nc.vector.tensor_sub(b_col, gnb_c[p], t_col)
                    # scale+shift in column halves so output DMA overlaps compute
                    for h0 in (0, 1152):
                        outp = gout.tile([128, 1152], F32, tag="outp", name="outp")
                        nc.vector.tensor_scalar(
                            outp,
                            Z[p][:, h0 : h0 + 1152],
                            a_col,
                            b_col,
                            op0=ALU.mult,
                            op1=ALU.add,
                        )
                        nc.sync.dma_start(
                            out=out_d[ts(p, 128), h0 : h0 + 1152], in_=outp
                        )

    nc.finalize()
    return nc


def _get_nc():
    if "nc" not in _BUILD_CACHE:
        _BUILD_CACHE["nc"] = _build_nc()
    return _BUILD_CACHE["nc"]


def _make_in_maps(inputs):
    import ml_dtypes

    f1 = np.ascontiguousarray(
        np.asarray(inputs["features1"], dtype=np.float32)
        .reshape(B, C, N)
        .astype(ml_dtypes.bfloat16)
    )
    f2 = np.ascontiguousarray(
        np.asarray(inputs["features2"], dtype=np.float32)
        .reshape(B, C, N)
        .astype(ml_dtypes.bfloat16)
    )

    def g(k):
        return np.asarray(inputs[k], dtype=np.float32)

    gsel = np.zeros((128, 2 * GROUPS), np.float32)
    gselT = np.zeros((GROUPS, 2 * 128), np.float32)
    for t in range(2):
        for gl in range(4):
            grp = 4 * t + gl
            gsel[gl * 32 : (gl + 1) * 32, GROUPS * t + grp] = 1.0
            gselT[grp, 128 * t + gl * 32 : 128 * t + (gl + 1) * 32] = 1.0
    shared = {
        "qwT": np.ascontiguousarray(g("q_w").T.astype(ml_dtypes.bfloat16)),
        "kwT": np.ascontiguousarray(g("k_w").T.astype(ml_dtypes.bfloat16)),
        "vwT": np.ascontiguousarray(g("v_w").T.astype(ml_dtypes.bfloat16)),
        "owT": np.ascontiguousarray(g("o_w").T),
        "q_b": g("q_b"),
        "k_b": g("k_b"),
        "vb_bcast": np.ascontiguousarray(np.tile(g("v_b")[None, :], (128, 1))),
        "o_b": g("o_b"),
        "gn_w": g("gn_w"),
        "gn_b": g("gn_b"),
        "gsel": gsel,
        "gselT": gselT,
        "ones_bf": np.ones((128, 1), ml_dtypes.bfloat16),
        "ones_row_r": np.ones((1, 128), np.float32),
    }
    return [{"features1": f1[i], "features2": f2[i], **shared} for i in range(B)]


def run(inputs, trace=False):
    from concourse.bass_utils import run_bass_kernel_spmd

    nc = _get_nc()
    in_maps = _make_in_maps(inputs)
    res = run_bass_kernel_spmd(nc, in_maps, core_ids=list(range(B)), trace=trace)
    out = np.stack([np.asarray(res.results[i]["out"]) for i in range(B)])
    return out.reshape(B, O, 48, 48).astype(np.float32), res


def kernel(**inputs):
    out, _ = run(inputs, trace=False)
    return out


# revision 19
# speedup vs baseline: 1.5144x; 1.0413x over previous
"""Trainium2 Bass kernel for CrossAttentionFusion.

Reference computation (per batch b):
    Q = q_w @ f1 + q_b          (O, N)   f1 = features1[b] as (C, N)
    K = k_w @ f2 + k_b          (O, N)
    V = v_w @ f2 + v_b          -> used as (N, O)
    A = softmax(Q^T K / sqrt(O))  over keys          (N, N)
    att = A @ V                  (N, O)
    Z = o_w @ att^T + o_b        (O, N)
    out = GroupNorm(8 groups over O, spatial N)(Z) * gn_w + gn_b

Sharding: pure data-parallel, batch b -> NeuronCore b (B=8, 8 cores).

Layout trick: scores are computed transposed (S' = K^T Q in [nk, nq] tiles)
so the exp output P' feeds the A@V matmul directly (lhsT = V tile, rhs = P')
with zero on-chip transposes of the big attention matrix. Softmax needs no
max-subtraction: scores ~ N(0,1), exp stays well inside fp32 range.
Denominators (cross-partition sums of P') come from a pairwise bf16 DVE tree
plus one ones-vector matmul; 1/denom is computed with the fast approximate
reciprocal and broadcast across partitions by a stride-0 DMA.
QK^T / projections run in float32r (fp32 storage, 1 cycle/row at free>=256);
P' and V are bf16 (DVE 4x mode, PE fast-weight-load). Weight matrices are
transposed host-side, so no on-chip weight transposes are needed.
"""

import numpy as np

B = 8
C = 256
O = 256
N = 2304
NKT = 18  # key tiles of 128
BIG = [(0, 1024), (1024, 1024), (2048, 256)]  # query chunks
GROUPS = 8
EPS = 1e-5
SCALE = float(O) ** -0.5

_BUILD_CACHE = {}


def _subs(jw):
    return [(s, min(512, jw - s)) for s in range(0, jw, 512)]


def _build_nc():
    import concourse.mybir as mybir
    import concourse.tile as tile
    from concourse import bacc
    from concourse.bass import ts

    F32 = mybir.dt.float32
    F32R = mybir.dt.float32r
    BF16 = mybir.dt.bfloat16
    AF = mybir.ActivationFunctionType
    ALU = mybir.AluOpType
    AX = mybir.AxisListType

    nc = bacc.Bacc("TRN2", target_bir_lowering=False)

    f1_d = nc.dram_tensor("features1", [C, N], BF16, kind="ExternalInput")
    f2_d = nc.dram_tensor("features2", [C, N], BF16, kind="ExternalInput")
    # host-pre-transposed weights
    qwT_d = nc.dram_tensor("qwT", [C, O], BF16, kind="ExternalInput")
    kwT_d = nc.dram_tensor("kwT", [C, O], BF16, kind="ExternalInput")
    vwT_d = nc.dram_tensor("vwT", [C, O], BF16, kind="ExternalInput")
    owT_d = nc.dram_tensor("owT", [O, O], F32R, kind="ExternalInput")
    qb_d = nc.dram_tensor("q_b", [O], F32, kind="ExternalInput")
    kb_d = nc.dram_tensor("k_b", [O], F32, kind="ExternalInput")
    vbb_d = nc.dram_tensor("vb_bcast", [128, O], F32, kind="ExternalInput")
    ob_d = nc.dram_tensor("o_b", [O], F32, kind="ExternalInput")
    gnw_d = nc.dram_tensor("gn_w", [O], F32, kind="ExternalInput")
    gnb_d = nc.dram_tensor("gn_b", [O], F32, kind="ExternalInput")
    gsel_d = nc.dram_tensor("gsel", [128, 2 * GROUPS], F32, kind="ExternalInput")
    gselT_d = nc.dram_tensor("gselT", [GROUPS, 2 * 128], F32, kind="ExternalInput")
    onesb_d = nc.dram_tensor("ones_bf", [128, 1], BF16, kind="ExternalInput")
    onesr_d = nc.dram_tensor("ones_row_r", [1, 128], F32R, kind="ExternalInput")
    out_d = nc.dram_tensor("out", [O, N], F32, kind="ExternalOutput")

    with tile.TileContext(nc) as tc:
        with (
            tc.tile_pool(name="consts", bufs=1) as consts,
            tc.tile_pool(name="weights", bufs=1) as wpool,
            tc.tile_pool(name="acts", bufs=1) as apool,
        ):

            # ---- constants ----
            zero_col = consts.tile([128, 1], F32, name="zero_col")
            nc.vector.memset(zero_col, 0.0)
            ones_bf = consts.tile([128, 1], BF16, name="ones_bf")
            nc.gpsimd.dma_start(out=ones_bf, in_=onesb_d[:, :])
            ones_row = consts.tile([1, 128], F32R, name="ones_row")
            nc.gpsimd.dma_start(out=ones_row, in_=onesr_d[:, :])
            vb_bc = consts.tile([128, O], F32, name="vb_bc")
            nc.gpsimd.dma_start(out=vb_bc, in_=vbb_d[:, :])
            gsel = consts.tile([128, 2 * GROUPS], F32, name="gsel")
            nc.gpsimd.dma_start(out=gsel, in_=gsel_d[:, :])
            gselT = consts.tile([GROUPS, 2 * 128], F32, name="gselT")
            nc.gpsimd.dma_start(out=gselT, in_=gselT_d[:, :])

            qb_c, kb_c, ob_c, gnw_c, gnb_c = [], [], [], [], []
            for t in range(2):
                for lst, src, nm in (
                    (qb_c, qb_d, "qb"),
                    (kb_c, kb_d, "kb"),
                    (ob_c, ob_d, "ob"),
                    (gnw_c, gnw_d, "gnw"),
                    (gnb_c, gnb_d, "gnb"),
                ):
                    col = consts.tile([128, 1], F32, name=f"{nm}{t}")
                    nc.gpsimd.dma_start(out=col, in_=src[ts(t, 128)].unsqueeze(1))
                    lst.append(col)

            # ---- persistent weights / activations ----
            qwT = [wpool.tile([128, O], BF16, name=f"qwT{t}") for t in range(2)]
            kwT = [wpool.tile([128, O], BF16, name=f"kwT{t}") for t in range(2)]
            vwT = [wpool.tile([128, O], BF16, name=f"vwT{t}") for t in range(2)]
            owT = [wpool.tile([128, O], F32R, name=f"owT{t}") for t in range(2)]
            for wt, wd in ((kwT, kwT_d), (vwT, vwT_d), (qwT, qwT_d), (owT, owT_d)):
                for t in range(2):
                    nc.scalar.dma_start(out=wt[t], in_=wd[ts(t, 128), :])
            Q = [apool.tile([128, N], BF16, name=f"Q{t}") for t in range(2)]
            K = [apool.tile([128, N], BF16, name=f"K{t}") for t in range(2)]
            V = [apool.tile([128, O], BF16, name=f"V{k}") for k in range(NKT)]
            Z = [apool.tile([128, N], F32, name=f"Z{t}") for t in range(2)]
            # running GroupNorm stats per p-tile: col0 = sum, col1 = sumsq
            st_sums = [apool.tile([128, 2], F32, name=f"st{t}") for t in range(2)]
            for t in range(2):
                nc.vector.memset(st_sums[t], 0.0)

            # ---- phase 1: feature loads + Q/K/V projections.
            # Feature tiles live only for this phase. DMA order (one FIFO
            # ring) interleaves f2/f1 halves; projection program order
            # matches arrival so the PE never stalls long.
            with (
                tc.tile_pool(name="feat", bufs=1) as fpool,
                tc.tile_pool(name="pps", bufs=4, space="PSUM") as pps,
            ):
                f1sb = [fpool.tile([128, N], BF16, name=f"f1sb{t}") for t in range(2)]
                f2sb = [fpool.tile([128, N], BF16, name=f"f2sb{t}") for t in range(2)]
                for c0 in (0, 1152):
                    for sb, dr in ((f2sb, f2_d), (f1sb, f1_d)):
                        for t in range(2):
                            nc.sync.dma_start(
                                out=sb[t][:, c0 : c0 + 1152],
                                in_=dr[ts(t, 128), c0 : c0 + 1152],
                            )

                def k_v_chunk(j0, jw):
                    jsl = slice(j0, j0 + jw)
                    for t in range(2):
                        kp = pps.tile([128, jw], F32, tag="pp", name="kp")
                        nc.tensor.matmul(
                            kp, kwT[0][:, ts(t, 128)], f2sb[0][:, jsl],
                            start=True, stop=False,
                        )
                        nc.tensor.matmul(
                            kp, kwT[1][:, ts(t, 128)], f2sb[1][:, jsl],
                            start=False, stop=True,
                        )
                        nc.vector.tensor_scalar_add(K[t][:, jsl], kp, kb_c[t])
                    for s in range(jw // 128):
                        nk = j0 // 128 + s
                        vp = pps.tile([128, O], F32, tag="pp", name="vp")
                        nc.tensor.matmul(
                            vp, f2sb[0][:, ts(nk, 128)], vwT[0], start=True, stop=False
                        )
                        nc.tensor.matmul(
                            vp, f2sb[1][:, ts(nk, 128)], vwT[1], start=False, stop=True
                        )
                        nc.vector.tensor_add(V[nk], vp, vb_bc)

                def q_chunk(j0, jw):
                    jsl = slice(j0, j0 + jw)
                    for t in range(2):
                        qp = pps.tile([128, jw], F32, tag="pp", name="qp")
                        nc.tensor.matmul(
                            qp, qwT[0][:, ts(t, 128)], f1sb[0][:, jsl],
                            start=True, stop=False,
                        )
                        nc.tensor.matmul(
                            qp, qwT[1][:, ts(t, 128)], f1sb[1][:, jsl],
                            start=False, stop=True,
                        )
                        nc.vector.tensor_scalar_add(Q[t][:, jsl], qp, qb_c[t])

                for j0, jw in ((0, 512), (512, 512)):
                    k_v_chunk(j0, jw)
                for j0, jw in ((0, 512), (512, 512)):
                    q_chunk(j0, jw)
                for j0, jw in ((1024, 512), (1536, 512), (2048, 256)):
                    k_v_chunk(j0, jw)
                for j0, jw in ((1024, 512), (1536, 512), (2048, 256)):
                    q_chunk(j0, jw)

            # ---- phase 2: attention + output projection, per query chunk ----
            with (
                tc.tile_pool(name="ppool", bufs=2) as ppool,
                tc.tile_pool(name="tpool", bufs=1) as tpool,
                tc.tile_pool(name="sbm", bufs=2) as sbm,
                tc.tile_pool(name="sps", bufs=2, space="PSUM") as sps,
                tc.tile_pool(name="ops", bufs=2, space="PSUM") as ops,
                tc.tile_pool(name="zps", bufs=1, space="PSUM") as zps,
                tc.tile_pool(name="dps", bufs=1, space="PSUM") as dps,
            ):
                for j0, jw in BIG:
                    subs = _subs(jw)
                    # scores (transposed) + exp, batched over the whole chunk
                    P = []
                    for nk in range(NKT):
                        sp = sps.tile([128, 1024], F32, tag="sp", name="sp")
                        for s0, sw in subs:
                            nc.tensor.matmul(
                                sp[:, s0 : s0 + sw],
                                K[0][:, ts(nk, 128)],
                                Q[0][:, j0 + s0 : j0 + s0 + sw],
                                start=True,
                                stop=False,
                            )
                            nc.tensor.matmul(
                                sp[:, s0 : s0 + sw],
                                K[1][:, ts(nk, 128)],
                                Q[1][:, j0 + s0 : j0 + s0 + sw],
                                start=False,
                                stop=True,
                            )
                        pt = ppool.tile([128, 1024], BF16, tag=f"p{nk}", name=f"pt{nk}")
                        nc.scalar.activation(
                            pt[:, :jw], sp[:, :jw], AF.Exp, bias=zero_col, scale=SCALE
                        )
                        P.append(pt)
                    # denominator: pairwise bf16 tree over the 18 P' tiles
                    tr = [
                        tpool.tile([128, 1024], BF16, tag=f"tr{i}", name=f"tr{i}")
                        for i in range(9)
                    ]
                    for i in range(9):
                        nc.vector.tensor_add(
                            tr[i][:, :jw], P[2 * i][:, :jw], P[2 * i + 1][:, :jw]
                        )
                    for i in range(4):
                        nc.vector.tensor_add(
                            tr[2 * i][:, :jw], tr[2 * i][:, :jw], tr[2 * i + 1][:, :jw]
                        )
                    nc.vector.tensor_add(tr[0][:, :jw], tr[0][:, :jw], tr[2][:, :jw])
                    nc.vector.tensor_add(tr[4][:, :jw], tr[4][:, :jw], tr[6][:, :jw])
                    nc.vector.tensor_add(tr[0][:, :jw], tr[0][:, :jw], tr[4][:, :jw])
                    nc.vector.tensor_add(tr[0][:, :jw], tr[0][:, :jw], tr[8][:, :jw])
                    for s0, sw in subs:
                        ssl = slice(s0, s0 + sw)
                        dn = dps.tile([1, 512], F32, tag="d", name="dn")
                        nc.tensor.matmul(
                            dn[:, :sw], ones_bf, tr[0][:, ssl], start=True, stop=True
                        )
                        dnr = sbm.tile([1, 512], F32R, tag="dnr", name="dnr")
                        nc.scalar.activation(dnr[:, :sw], dn[:, :sw], AF.Copy)
                        bc = dps.tile([128, 512], F32, tag="d", name="bc")
                        nc.tensor.matmul(
                            bc[:, :sw], ones_row, dnr[:, :sw], start=True, stop=True
                        )
                        bcs = sbm.tile([128, 512], F32, tag="bcs", name="bcs")
                        nc.vector.reciprocal_approx_fast(bcs[:, :sw], bc[:, :sw])
                        # att^T sub-chunk = (V^T P') * (1/denom) : [O, sw]
                        ATs = []
                        for o in range(2):
                            op = ops.tile([128, 512], F32, tag="op", name="op")
                            for nk in range(NKT):
                                nc.tensor.matmul(
                                    op[:, :sw],
                                    V[nk][:, ts(o, 128)],
                                    P[nk][:, ssl],
                                    start=(nk == 0),
                                    stop=(nk == NKT - 1),
                                )
                            at = sbm.tile([128, 512], F32R, tag=f"at{o}", name=f"at{o}")
                            nc.vector.tensor_mul(at[:, :sw], op[:, :sw], bcs[:, :sw])
                            ATs.append(at)
                        # output projection sub-chunk: Z[p, sw]
                        zsl = slice(j0 + s0, j0 + s0 + sw)
                        for p in range(2):
                            zp = zps.tile([128, 512], F32, tag="zp", name="zp")
                            nc.tensor.matmul(
                                zp[:, :sw],
                                owT[0][:, ts(p, 128)],
                                ATs[0][:, :sw],
                                start=True,
                                stop=False,
                            )
                            nc.tensor.matmul(
                                zp[:, :sw],
                                owT[1][:, ts(p, 128)],
                                ATs[1][:, :sw],
                                start=False,
                                stop=True,
                            )
                            # evacuate + bias; accum_out gives GN row-sums free
                            part = sbm.tile(
                                [128, 2], F32, tag=f"part{p}", name=f"part{p}"
                            )
                            nc.vector.tensor_scalar(
                                Z[p][:, zsl],
                                zp[:, :sw],
                                ob_c[p],
                                0.0,
                                op0=ALU.add,
                                op1=ALU.add,
                                accum_out=part[:, 0:1],
                            )
                            sqs = sbm.tile([128, 512], F32, tag="sqs", name="sqs")
                            nc.scalar.activation(
                                sqs[:, :sw],
                                Z[p][:, zsl],
                                AF.Square,
                                bias=zero_col,
                                accum_out=part[:, 1:2],
                            )
                            nc.vector.tensor_add(st_sums[p], st_sums[p], part)

            # ---- phase 3: GroupNorm finalization ----
            with (
                tc.tile_pool(name="gns", bufs=2) as gns,
                tc.tile_pool(name="gout", bufs=2) as gout,
                tc.tile_pool(name="gps", bufs=2, space="PSUM") as gps,
            ):
                gst = gps.tile([GROUPS, 2], F32, tag="gst", name="gst")
                nc.tensor.matmul(
                    gst, gsel[:, 0:GROUPS], st_sums[0], start=True, stop=False
                )
                nc.tensor.matmul(
                    gst,
                    gsel[:, GROUPS : 2 * GROUPS],
                    st_sums[1],
                    start=False,
                    stop=True,
                )
                # per-group mean / rstd on 8 partitions
                mv = gns.tile([GROUPS, 8], F32, tag="mv", name="mv")
                inv_cnt = 1.0 / (32.0 * N)
                nc.vector.tensor_scalar_mul(mv[:, 0:1], gst[:, 0:1], inv_cnt)  # mean
                nc.vector.tensor_scalar_mul(mv[:, 1:2], gst[:, 1:2], inv_cnt)  # E[x^2]
                nc.vector.tensor_mul(mv[:, 2:3], mv[:, 0:1], mv[:, 0:1])  # mean^2
                nc.vector.tensor_sub(mv[:, 3:4], mv[:, 1:2], mv[:, 2:3])  # var
                nc.vector.tensor_scalar_add(mv[:, 3:4], mv[:, 3:4], EPS)  # var+eps
                nc.scalar.activation(
                    mv[:, 4:5], mv[:, 3:4], AF.Sqrt, bias=zero_col[0:GROUPS, :]
                )
                nc.vector.reciprocal(mv[:, 5:6], mv[:, 4:5])  # y0 ~ rsqrt
                # one Newton step: y1 = y0 * (1.5 - 0.5*(var+eps)*y0^2)
                nc.vector.tensor_mul(mv[:, 6:7], mv[:, 5:6], mv[:, 5:6])  # y0^2
                nc.vector.tensor_mul(mv[:, 6:7], mv[:, 6:7], mv[:, 3:4])  # v*y0^2
                nc.vector.tensor_scalar(
                    mv[:, 6:7], mv[:, 6:7], -0.5, 1.5, op0=ALU.mult, op1=ALU.add
                )
                nc.vector.tensor_mul(mv[:, 7:8], mv[:, 5:6], mv[:, 6:7])  # rstd
                gm2 = gns.tile([GROUPS, 2], F32, tag="gm2", name="gm2")
                nc.vector.tensor_copy(gm2[:, 0:1], mv[:, 0:1])
                nc.vector.tensor_copy(gm2[:, 1:2], mv[:, 7:8])
                for p in range(2):
                    pst = gps.tile([128, 2], F32, tag="pst", name="pst")
                    nc.tensor.matmul(
                        pst, gselT[:, ts(p, 128)], gm2, start=True, stop=True
                    )
                    a_col = gns.tile([128, 1], F32, tag="a_col", name="a_col")
                    nc.vector.tensor_mul(a_col, pst[:, 1:2], gnw_c[p])
                    t_col = gns.tile([128, 1], F32, tag="t_col", name="t_col")
                    nc.vector.tensor_mul(t_col, pst[:, 0:1], a_col)
                    b_col = gns.tile([128, 1], F32, tag="b_col", name="b_col")
                    nc.vector.tensor_sub(b_col, gnb_c[p], t_col)
                    # scale+shift in column halves so output DMA overlaps compute
                    for h0 in (0, 1152):
                        outp = gout.tile([128, 1152], F32, tag="outp", name="outp")
                        nc.vector.tensor_scalar(
                            outp,
                            Z[p][:, h0 : h0 + 1152],
                            a_col,
                            b_col,
                            op0=ALU.mult,
                            op1=ALU.add,
                        )
                        nc.sync.dma_start(
                            out=out_d[ts(p, 128), h0 : h0 + 1152], in_=outp
                        )

    nc.finalize()
    return nc


def _get_nc():
    if "nc" not in _BUILD_CACHE:
        _BUILD_CACHE["nc"] = _build_nc()
    return _BUILD_CACHE["nc"]


def _make_in_maps(inputs):
    import ml_dtypes

    f1 = np.ascontiguousarray(
        np.asarray(inputs["features1"], dtype=np.float32)
        .reshape(B, C, N)
        .astype(ml_dtypes.bfloat16)
    )
    f2 = np.ascontiguousarray(
        np.asarray(inputs["features2"], dtype=np.float32)
        .reshape(B, C, N)
        .astype(ml_dtypes.bfloat16)
    )

    def g(k):
        return np.asarray(inputs[k], dtype=np.float32)

    gsel = np.zeros((128, 2 * GROUPS), np.float32)
    gselT = np.zeros((GROUPS, 2 * 128), np.float32)
    for t in range(2):
        for gl in range(4):
            grp = 4 * t + gl
            gsel[gl * 32 : (gl + 1) * 32, GROUPS * t + grp] = 1.0
            gselT[grp, 128 * t + gl * 32 : 128 * t + (gl + 1) * 32] = 1.0
    shared = {
        "qwT": np.ascontiguousarray(g("q_w").T.astype(ml_dtypes.bfloat16)),
        "kwT": np.ascontiguousarray(g("k_w").T.astype(ml_dtypes.bfloat16)),
        "vwT": np.ascontiguousarray(g("v_w").T.astype(ml_dtypes.bfloat16)),
        "owT": np.ascontiguousarray(g("o_w").T),
        "q_b": g("q_b"),
        "k_b": g("k_b"),
        "vb_bcast": np.ascontiguousarray(np.tile(g("v_b")[None, :], (128, 1))),
        "o_b": g("o_b"),
        "gn_w": g("gn_w"),
        "gn_b": g("gn_b"),
        "gsel": gsel,
        "gselT": gselT,
        "ones_bf": np.ones((128, 1), ml_dtypes.bfloat16),
        "ones_row_r": np.ones((1, 128), np.float32),
    }
    return [{"features1": f1[i], "features2": f2[i], **shared} for i in range(B)]


def run(inputs, trace=False):
    from concourse.bass_utils import run_bass_kernel_spmd

    nc = _get_nc()
    in_maps = _make_in_maps(inputs)
    res = run_bass_kernel_spmd(nc, in_maps, core_ids=list(range(B)), trace=trace)
    out = np.stack([np.asarray(res.results[i]["out"]) for i in range(B)])
    return out.reshape(B, O, 48, 48).astype(np.float32), res


def kernel(**inputs):
    out, _ = run(inputs, trace=False)
    return out


# revision 20
# speedup vs baseline: 1.5174x; 1.0020x over previous
"""Trainium2 Bass kernel for CrossAttentionFusion.

Reference computation (per batch b):
    Q = q_w @ f1 + q_b          (O, N)   f1 = features1[b] as (C, N)
    K = k_w @ f2 + k_b          (O, N)
    V = v_w @ f2 + v_b          -> used as (N, O)
    A = softmax(Q^T K / sqrt(O))  over keys          (N, N)
    att = A @ V                  (N, O)
    Z = o_w @ att^T + o_b        (O, N)
    out = GroupNorm(8 groups over O, spatial N)(Z) * gn_w + gn_b

Sharding: pure data-parallel, batch b -> NeuronCore b (B=8, 8 cores).

Layout trick: scores are computed transposed (S' = K^T Q in [nk, nq] tiles)
so the exp output P' feeds the A@V matmul directly (lhsT = V tile, rhs = P')
with zero on-chip transposes of the big attention matrix. Softmax needs no
max-subtraction: scores ~ N(0,1), exp stays well inside fp32 range.
Denominators (cross-partition sums of P') come from a pairwise bf16 DVE tree
plus one ones-vector matmul; 1/denom is computed with the fast approximate
reciprocal and broadcast across partitions by a stride-0 DMA.
QK^T / projections run in float32r (fp32 storage, 1 cycle/row at free>=256);
P' and V are bf16 (DVE 4x mode, PE fast-weight-load). Weight matrices are
transposed host-side, so no on-chip weight transposes are needed.
"""

import numpy as np

B = 8
C = 256
O = 256
N = 2304
NKT = 18  # key tiles of 128
BIG = [(0, 1024), (1024, 1024), (2048, 256)]  # query chunks
GROUPS = 8
EPS = 1e-5
SCALE = float(O) ** -0.5

_BUILD_CACHE = {}


def _subs(jw):
    return [(s, min(512, jw - s)) for s in range(0, jw, 512)]


def _build_nc():
    import concourse.mybir as mybir
    import concourse.tile as tile
    from concourse import bacc
    from concourse.bass import ts

    F32 = mybir.dt.float32
    F32R = mybir.dt.float32r
    BF16 = mybir.dt.bfloat16
    AF = mybir.ActivationFunctionType
    ALU = mybir.AluOpType
    AX = mybir.AxisListType

    nc = bacc.Bacc("TRN2", target_bir_lowering=False)

    f1_d = nc.dram_tensor("features1", [C, N], BF16, kind="ExternalInput")
    f2_d = nc.dram_tensor("features2", [C, N], BF16, kind="ExternalInput")
    # host-pre-transposed weights
    qwT_d = nc.dram_tensor("qwT", [C, O], BF16, kind="ExternalInput")
    kwT_d = nc.dram_tensor("kwT", [C, O], BF16, kind="ExternalInput")
    vwT_d = nc.dram_tensor("vwT", [C, O], BF16, kind="ExternalInput")
    owT_d = nc.dram_tensor("owT", [O, O], F32R, kind="ExternalInput")
    qb_d = nc.dram_tensor("q_b", [O], F32, kind="ExternalInput")
    kb_d = nc.dram_tensor("k_b", [O], F32, kind="ExternalInput")
    vbb_d = nc.dram_tensor("vb_bcast", [128, O], F32, kind="ExternalInput")
    ob_d = nc.dram_tensor("o_b", [O], F32, kind="ExternalInput")
    gnw_d = nc.dram_tensor("gn_w", [O], F32, kind="ExternalInput")
    gnb_d = nc.dram_tensor("gn_b", [O], F32, kind="ExternalInput")
    gsel_d = nc.dram_tensor("gsel", [128, 2 * GROUPS], F32, kind="ExternalInput")
    gselT_d = nc.dram_tensor("gselT", [GROUPS, 2 * 128], F32, kind="ExternalInput")
    onesb_d = nc.dram_tensor("ones_bf", [128, 1], BF16, kind="ExternalInput")
    onesr_d = nc.dram_tensor("ones_row_r", [1, 128], F32R, kind="ExternalInput")
    out_d = nc.dram_tensor("out", [O, N], F32, kind="ExternalOutput")

    with tile.TileContext(nc) as tc:
        with (
            tc.tile_pool(name="consts", bufs=1) as consts,
            tc.tile_pool(name="weights", bufs=1) as wpool,
            tc.tile_pool(name="acts", bufs=1) as apool,
        ):

            # ---- constants ----
            zero_col = consts.tile([128, 1], F32, name="zero_col")
            nc.vector.memset(zero_col, 0.0)
            ones_bf = consts.tile([128, 1], BF16, name="ones_bf")
            nc.gpsimd.dma_start(out=ones_bf, in_=onesb_d[:, :])
            ones_row = consts.tile([1, 128], F32R, name="ones_row")
            nc.gpsimd.dma_start(out=ones_row, in_=onesr_d[:, :])
            vb_bc = consts.tile([128, O], F32, name="vb_bc")
            nc.gpsimd.dma_start(out=vb_bc, in_=vbb_d[:, :])
            gsel = consts.tile([128, 2 * GROUPS], F32, name="gsel")
            nc.gpsimd.dma_start(out=gsel, in_=gsel_d[:, :])
            gselT = consts.tile([GROUPS, 2 * 128], F32, name="gselT")
            nc.gpsimd.dma_start(out=gselT, in_=gselT_d[:, :])

            qb_c, kb_c, ob_c, gnw_c, gnb_c = [], [], [], [], []
            for t in range(2):
                for lst, src, nm in (
                    (qb_c, qb_d, "qb"),
                    (kb_c, kb_d, "kb"),
                    (ob_c, ob_d, "ob"),
                    (gnw_c, gnw_d, "gnw"),
                    (gnb_c, gnb_d, "gnb"),
                ):
                    col = consts.tile([128, 1], F32, name=f"{nm}{t}")
                    nc.gpsimd.dma_start(out=col, in_=src[ts(t, 128)].unsqueeze(1))
                    lst.append(col)

            # ---- persistent weights / activations ----
            qwT = [wpool.tile([128, O], BF16, name=f"qwT{t}") for t in range(2)]
            kwT = [wpool.tile([128, O], BF16, name=f"kwT{t}") for t in range(2)]
            vwT = [wpool.tile([128, O], BF16, name=f"vwT{t}") for t in range(2)]
            owT = [wpool.tile([128, O], F32R, name=f"owT{t}") for t in range(2)]
            for wt, wd in ((kwT, kwT_d), (vwT, vwT_d), (qwT, qwT_d), (owT, owT_d)):
                for t in range(2):
                    nc.scalar.dma_start(out=wt[t], in_=wd[ts(t, 128), :])
            Q = [apool.tile([128, N], BF16, name=f"Q{t}") for t in range(2)]
            K = [apool.tile([128, N], BF16, name=f"K{t}") for t in range(2)]
            V = [apool.tile([128, O], BF16, name=f"V{k}") for k in range(NKT)]
            Z = [apool.tile([128, N], F32, name=f"Z{t}") for t in range(2)]
            # running GroupNorm stats per p-tile: col0 = sum, col1 = sumsq
            st_sums = [apool.tile([128, 2], F32, name=f"st{t}") for t in range(2)]
            for t in range(2):
                nc.vector.memset(st_sums[t], 0.0)

            # ---- phase 1: feature loads + Q/K/V projections.
            # Feature tiles live only for this phase. DMA order (one FIFO
            # ring) interleaves f2/f1 halves; projection program order
            # matches arrival so the PE never stalls long.
            with (
                tc.tile_pool(name="feat", bufs=1) as fpool,
                tc.tile_pool(name="pps", bufs=4, space="PSUM") as pps,
            ):
                f1sb = [fpool.tile([128, N], BF16, name=f"f1sb{t}") for t in range(2)]
                f2sb = [fpool.tile([128, N], BF16, name=f"f2sb{t}") for t in range(2)]
                for c0 in (0, 576, 1152, 1728):
                    for sb, dr in ((f2sb, f2_d), (f1sb, f1_d)):
                        for t in range(2):
                            nc.sync.dma_start(
                                out=sb[t][:, c0 : c0 + 576],
                                in_=dr[ts(t, 128), c0 : c0 + 576],
                            )

                def k_v_chunk(j0, jw):
                    jsl = slice(j0, j0 + jw)
                    for t in range(2):
                        kp = pps.tile([128, jw], F32, tag="pp", name="kp")
                        nc.tensor.matmul(
                            kp, kwT[0][:, ts(t, 128)], f2sb[0][:, jsl],
                            start=True, stop=False,
                        )
                        nc.tensor.matmul(
                            kp, kwT[1][:, ts(t, 128)], f2sb[1][:, jsl],
                            start=False, stop=True,
                        )
                        nc.vector.tensor_scalar_add(K[t][:, jsl], kp, kb_c[t])
                    for s in range(jw // 128):
                        nk = j0 // 128 + s
                        vp = pps.tile([128, O], F32, tag="pp", name="vp")
                        nc.tensor.matmul(
                            vp, f2sb[0][:, ts(nk, 128)], vwT[0], start=True, stop=False
                        )
                        nc.tensor.matmul(
                            vp, f2sb[1][:, ts(nk, 128)], vwT[1], start=False, stop=True
                        )
                        nc.vector.tensor_add(V[nk], vp, vb_bc)

                def q_chunk(j0, jw):
                    jsl = slice(j0, j0 + jw)
                    for t in range(2):
                        qp = pps.tile([128, jw], F32, tag="pp", name="qp")
                        nc.tensor.matmul(
                            qp, qwT[0][:, ts(t, 128)], f1sb[0][:, jsl],
                            start=True, stop=False,
                        )
                        nc.tensor.matmul(
                            qp, qwT[1][:, ts(t, 128)], f1sb[1][:, jsl],
                            start=False, stop=True,
                        )
                        nc.vector.tensor_scalar_add(Q[t][:, jsl], qp, qb_c[t])

                for j0, jw in ((0, 512), (512, 512)):
                    k_v_chunk(j0, jw)
                for j0, jw in ((0, 512), (512, 512)):
                    q_chunk(j0, jw)
                for j0, jw in ((1024, 512), (1536, 512), (2048, 256)):
                    k_v_chunk(j0, jw)
                for j0, jw in ((1024, 512), (1536, 512), (2048, 256)):
                    q_chunk(j0, jw)

            # ---- phase 2: attention + output projection, per query chunk ----
            with (
                tc.tile_pool(name="ppool", bufs=2) as ppool,
                tc.tile_pool(name="tpool", bufs=1) as tpool,
                tc.tile_pool(name="sbm", bufs=2) as sbm,
                tc.tile_pool(name="sps", bufs=2, space="PSUM") as sps,
                tc.tile_pool(name="ops", bufs=2, space="PSUM") as ops,
                tc.tile_pool(name="zps", bufs=1, space="PSUM") as zps,
                tc.tile_pool(name="dps", bufs=1, space="PSUM") as dps,
            ):
                for j0, jw in BIG:
                    subs = _subs(jw)
                    # scores (transposed) + exp, batched over the whole chunk
                    P = []
                    for nk in range(NKT):
                        sp = sps.tile([128, 1024], F32, tag="sp", name="sp")
                        for s0, sw in subs:
                            nc.tensor.matmul(
                                sp[:, s0 : s0 + sw],
                                K[0][:, ts(nk, 128)],
                                Q[0][:, j0 + s0 : j0 + s0 + sw],
                                start=True,
                                stop=False,
                            )
                            nc.tensor.matmul(
                                sp[:, s0 : s0 + sw],
                                K[1][:, ts(nk, 128)],
                                Q[1][:, j0 + s0 : j0 + s0 + sw],
                                start=False,
                                stop=True,
                            )
                        pt = ppool.tile([128, 1024], BF16, tag=f"p{nk}", name=f"pt{nk}")
                        nc.scalar.activation(
                            pt[:, :jw], sp[:, :jw], AF.Exp, bias=zero_col, scale=SCALE
                        )
                        P.append(pt)
                    # denominator: pairwise bf16 tree over the 18 P' tiles
                    tr = [
                        tpool.tile([128, 1024], BF16, tag=f"tr{i}", name=f"tr{i}")
                        for i in range(9)
                    ]
                    for i in range(9):
                        nc.vector.tensor_add(
                            tr[i][:, :jw], P[2 * i][:, :jw], P[2 * i + 1][:, :jw]
                        )
                    for i in range(4):
                        nc.vector.tensor_add(
                            tr[2 * i][:, :jw], tr[2 * i][:, :jw], tr[2 * i + 1][:, :jw]
                        )
                    nc.vector.tensor_add(tr[0][:, :jw], tr[0][:, :jw], tr[2][:, :jw])
                    nc.vector.tensor_add(tr[4][:, :jw], tr[4][:, :jw], tr[6][:, :jw])
                    nc.vector.tensor_add(tr[0][:, :jw], tr[0][:, :jw], tr[4][:, :jw])
                    nc.vector.tensor_add(tr[0][:, :jw], tr[0][:, :jw], tr[8][:, :jw])
                    for s0, sw in subs:
                        ssl = slice(s0, s0 + sw)
                        dn = dps.tile([1, 512], F32, tag="d", name="dn")
                        nc.tensor.matmul(
                            dn[:, :sw], ones_bf, tr[0][:, ssl], start=True, stop=True
                        )
                        dnr = sbm.tile([1, 512], F32R, tag="dnr", name="dnr")
                        nc.scalar.activation(dnr[:, :sw], dn[:, :sw], AF.Copy)
                        bc = dps.tile([128, 512], F32, tag="d", name="bc")
                        nc.tensor.matmul(
                            bc[:, :sw], ones_row, dnr[:, :sw], start=True, stop=True
                        )
                        bcs = sbm.tile([128, 512], F32, tag="bcs", name="bcs")
                        nc.vector.reciprocal_approx_fast(bcs[:, :sw], bc[:, :sw])
                        # att^T sub-chunk = (V^T P') * (1/denom) : [O, sw]
                        ATs = []
                        for o in range(2):
                            op = ops.tile([128, 512], F32, tag="op", name="op")
                            for nk in range(NKT):
                                nc.tensor.matmul(
                                    op[:, :sw],
                                    V[nk][:, ts(o, 128)],
                                    P[nk][:, ssl],
                                    start=(nk == 0),
                                    stop=(nk == NKT - 1),
                                )
                            at = sbm.tile([128, 512], F32R, tag=f"at{o}", name=f"at{o}")
                            nc.vector.tensor_mul(at[:, :sw], op[:, :sw], bcs[:, :sw])
                            ATs.append(at)
                        # output projection sub-chunk: Z[p, sw]
                        zsl = slice(j0 + s0, j0 + s0 + sw)
                        for p in range(2):
                            zp = zps.tile([128, 512], F32, tag="zp", name="zp")
                            nc.tensor.matmul(
                                zp[:, :sw],
                                owT[0][:, ts(p, 128)],
                                ATs[0][:, :sw],
                                start=True,
                                stop=False,
                            )
                            nc.tensor.matmul(
                                zp[:, :sw],
                                owT[1][:, ts(p, 128)],
                                ATs[1][:, :sw],
                                start=False,
                                stop=True,
                            )
                            # evacuate + bias; accum_out gives GN row-sums free
                            part = sbm.tile(
                                [128, 2], F32, tag=f"part{p}", name=f"part{p}"
                            )
                            nc.vector.tensor_scalar(
                                Z[p][:, zsl],
                                zp[:, :sw],
                                ob_c[p],
                                0.0,
                                op0=ALU.add,
                                op1=ALU.add,
                                accum_out=part[:, 0:1],
                            )
                            sqs = sbm.tile([128, 512], F32, tag="sqs", name="sqs")
                            nc.scalar.activation(
                                sqs[:, :sw],
                                Z[p][:, zsl],
                                AF.Square,
                                bias=zero_col,
                                accum_out=part[:, 1:2],
                            )
                            nc.vector.tensor_add(st_sums[p], st_sums[p], part)

            # ---- phase 3: GroupNorm finalization ----
            with (
                tc.tile_pool(name="gns", bufs=2) as gns,
                tc.tile_pool(name="gout", bufs=2) as gout,
                tc.tile_pool(name="gps", bufs=2, space="PSUM") as gps,
            ):
                gst = gps.tile([GROUPS, 2], F32, tag="gst", name="gst")
                nc.tensor.matmul(
                    gst, gsel[:, 0:GROUPS], st_sums[0], start=True, stop=False
                )
                nc.tensor.matmul(
                    gst,
                    gsel[:, GROUPS : 2 * GROUPS],
                    st_sums[1],
                    start=False,
                    stop=True,
                )
                # per-group mean / rstd on 8 partitions
                mv = gns.tile([GROUPS, 8], F32, tag="mv", name="mv")
                inv_cnt = 1.0 / (32.0 * N)
                nc.vector.tensor_scalar_mul(mv[:, 0:1], gst[:, 0:1], inv_cnt)  # mean
                nc.vector.tensor_scalar_mul(mv[:, 1:2], gst[:, 1:2], inv_cnt)  # E[x^2]
                nc.vector.tensor_mul(mv[:, 2:3], mv[:, 0:1], mv[:, 0:1])  # mean^2
                nc.vector.tensor_sub(mv[:, 3:4], mv[:, 1:2], mv[:, 2:3])  # var
                nc.vector.tensor_scalar_add(mv[:, 3:4], mv[:, 3:4], EPS)  # var+eps
                nc.scalar.activation(
                    mv[:, 4:5], mv[:, 3:4], AF.Sqrt, bias=zero_col[0:GROUPS, :]
                )
                nc.vector.reciprocal(mv[:, 5:6], mv[:, 4:5])  # y0 ~ rsqrt
                # one Newton step: y1 = y0 * (1.5 - 0.5*(var+eps)*y0^2)
                nc.vector.tensor_mul(mv[:, 6:7], mv[:, 5:6], mv[:, 5:6])  # y0^2
                nc.vector.tensor_mul(mv[:, 6:7], mv[:, 6:7], mv[:, 3:4])  # v*y0^2
                nc.vector.tensor_scalar(
                    mv[:, 6:7], mv[:, 6:7], -0.5, 1.5, op0=ALU.mult, op1=ALU.add
                )
                nc.vector.tensor_mul(mv[:, 7:8], mv[:, 5:6], mv[:, 6:7])  # rstd
                gm2 = gns.tile([GROUPS, 2], F32, tag="gm2", name="gm2")
                nc.vector.tensor_copy(gm2[:, 0:1], mv[:, 0:1])
                nc.vector.tensor_copy(gm2[:, 1:2], mv[:, 7:8])
                for p in range(2):
                    pst = gps.tile([128, 2], F32, tag="pst", name="pst")
                    nc.tensor.matmul(
                        pst, gselT[:, ts(p, 128)], gm2, start=True, stop=True
                    )
                    a_col = gns.tile([128, 1], F32, tag="a_col", name="a_col")
                    nc.vector.tensor_mul(a_col, pst[:, 1:2], gnw_c[p])
                    t_col = gns.tile([128, 1], F32, tag="t_col", name="t_col")
                    nc.vector.tensor_mul(t_col, pst[:, 0:1], a_col)
                    b_col = gns.tile([128, 1], F32, tag="b_col", name="b_col")
                    nc.vector.tensor_sub(b_col, gnb_c[p], t_col)
                    # scale+shift in column halves so output DMA overlaps compute
                    for h0 in (0, 1152):
                        outp = gout.tile([128, 1152], F32, tag="outp", name="outp")
                        nc.vector.tensor_scalar(
                            outp,
                            Z[p][:, h0 : h0 + 1152],
                            a_col,
                            b_col,
                            op0=ALU.mult,
                            op1=ALU.add,
                        )
                        nc.sync.dma_start(
                            out=out_d[ts(p, 128), h0 : h0 + 1152], in_=outp
                        )

    nc.finalize()
    return nc


def _get_nc():
    if "nc" not in _BUILD_CACHE:
        _BUILD_CACHE["nc"] = _build_nc()
    return _BUILD_CACHE["nc"]


def _make_in_maps(inputs):
    import ml_dtypes

    f1 = np.ascontiguousarray(
        np.asarray(inputs["features1"], dtype=np.float32)
        .reshape(B, C, N)
        .astype(ml_dtypes.bfloat16)
    )
    f2 = np.ascontiguousarray(
        np.asarray(inputs["features2"], dtype=np.float32)
        .reshape(B, C, N)
        .astype(ml_dtypes.bfloat16)
    )

    def g(k):
        return np.asarray(inputs[k], dtype=np.float32)

    gsel = np.zeros((128, 2 * GROUPS), np.float32)
    gselT = np.zeros((GROUPS, 2 * 128), np.float32)
    for t in range(2):
        for gl in range(4):
            grp = 4 * t + gl
            gsel[gl * 32 : (gl + 1) * 32, GROUPS * t + grp] = 1.0
            gselT[grp, 128 * t + gl * 32 : 128 * t + (gl + 1) * 32] = 1.0
    shared = {
        "qwT": np.ascontiguousarray(g("q_w").T.astype(ml_dtypes.bfloat16)),
        "kwT": np.ascontiguousarray(g("k_w").T.astype(ml_dtypes.bfloat16)),
        "vwT": np.ascontiguousarray(g("v_w").T.astype(ml_dtypes.bfloat16)),
        "owT": np.ascontiguousarray(g("o_w").T),
        "q_b": g("q_b"),
        "k_b": g("k_b"),
        "vb_bcast": np.ascontiguousarray(np.tile(g("v_b")[None, :], (128, 1))),
        "o_b": g("o_b"),
        "gn_w": g("gn_w"),
        "gn_b": g("gn_b"),
        "gsel": gsel,
        "gselT": gselT,
        "ones_bf": np.ones((128, 1), ml_dtypes.bfloat16),
        "ones_row_r": np.ones((1, 128), np.float32),
    }
    return [{"features1": f1[i], "features2": f2[i], **shared} for i in range(B)]


def run(inputs, trace=False):
    from concourse.bass_utils import run_bass_kernel_spmd

    nc = _get_nc()
    in_maps = _make_in_maps(inputs)
    res = run_bass_kernel_spmd(nc, in_maps, core_ids=list(range(B)), trace=trace)
    out = np.stack([np.asarray(res.results[i]["out"]) for i in range(B)])
    return out.reshape(B, O, 48, 48).astype(np.float32), res


def kernel(**inputs):
    out, _ = run(inputs, trace=False)
    return out
